# revision 13
# baseline (speedup 1.0000x reference)
"""Trainium2 Bass kernel for nn_ANEDecoder (Dia-style ANE decoder, 2 layers).

Sharding: tensor-parallel across 8 cores — 2 query heads + their (shared)
KV head per core for self-attn, 2 cross heads per core, FF/8 = 1024 MLP
hidden per core, all-reduce after o_proj / co_proj / wd.

v3 over v2:
- Post-scale RMSNorm: blocks matmul the *un-normalized* residual (XB, bf16)
  immediately after the all-reduce lands; the 1/rms factor is applied to the
  (much smaller) projection outputs, so the stats chain overlaps the GEMMs
  instead of serializing in front of them.
- Masked-chunk skipping: self-attention only touches the ceil((w+S)/128)
  cache chunks the causal mask can reach (3 of 12 for w=256); cross-attn
  only ceil(enc_len/128) chunks. Fully-masked chunks contribute exp(-3e4)=0
  exactly, so skipping is lossless.
- AR readback on the sync queue (gpsimd queue carries only collectives),
  residual add writes the bf16 GEMM input directly (f32 master copy updated
  off-path on gpsimd).
- Warmup collective issued as the first instruction so the one-time CC
  bring-up overlaps the prologue DMA.
"""
import os
import sys
import functools
from contextlib import ExitStack

sys.path.insert(0, "/opt/trn_rl_repo")

import numpy as np
import ml_dtypes

import concourse.bass as bass
import concourse.bacc as bacc
import concourse.mybir as mybir
import concourse.tile as tile
import concourse.masks as masks
from concourse.bass_utils import run_bass_kernel_spmd

BF = ml_dtypes.bfloat16
F32 = mybir.dt.float32
BF16 = mybir.dt.bfloat16
AF = mybir.ActivationFunctionType
ALU = mybir.AluOpType

# dims
B, D, S = 2, 2048, 128
A, T = 1536, 512
HQ, HKV, HD = 16, 4, 128
HC, HDC = 16, 128
FF, L = 8192, 2
EPS = 1e-5
NEG = -3e4

NCORES = 8
NDC = D // 128          # 16 d-chunks
QH = HQ // NCORES       # 2 query heads per core
CH = HC // NCORES       # 2 cross heads per core
FFS = FF // NCORES      # 1024 ff per core
NFC = FFS // 128        # 8 ff chunks

_exec_time_ns = None


def last_exec_time_ns():
    return _exec_time_ns


# ---------------------------------------------------------------- builder
@functools.lru_cache(maxsize=4)
def build_graph(w: int, nkc: int, ntck: int):
    KA = nkc * 128          # kept self-attn cache columns
    KT = ntck * 128         # kept cross-attn cache columns
    nc = bacc.Bacc()

    def par(name, shape, dt):
        return nc.declare_dram_parameter(name, list(shape), dt, isOutput=False)

    x_in = par("x_in", (D, B * S), F32)
    qw = par("qw", (L, D, QH * HD), BF16)
    kvw = par("kvw", (L, D, 2 * HD), BF16)
    ow = par("ow", (L, QH * HD, D), BF16)
    cqw = par("cqw", (L, D, CH * HDC), BF16)
    cow = par("cow", (L, CH * HDC, D), BF16)
    wgw = par("wgw", (L, D, FFS), BF16)
    wuw = par("wuw", (L, D, FFS), BF16)
    wdw = par("wdw", (L, FFS, D), BF16)
    kT = par("kT", (L, B, HD, KA), BF16)
    vT = par("vT", (L, B, KA, HD), BF16)
    ckT = par("ckT", (L, B, CH, HDC, KT), BF16)
    cvT = par("cvT", (L, B, CH, KT, HDC), BF16)
    sinT = par("sinT", (HD // 2, B * S), F32)
    cosT = par("cosT", (HD // 2, B * S), F32)
    smask = par("smask", (S, B * KA), BF16)
    cmask = par("cmask", (S, B * KT), BF16)
    fnw = par("fnw", (128, NDC), F32)
    out_ext = nc.declare_dram_parameter("out", [D, B * S], F32, isOutput=True)

    RG = [list(range(NCORES))]

    with tile.TileContext(nc) as tc, ExitStack() as es:
        persist = es.enter_context(tc.tile_pool(name="persist", bufs=1))
        cachep = es.enter_context(tc.tile_pool(name="cache", bufs=2))
        wbig = es.enter_context(tc.tile_pool(name="wbig", bufs=2))
        wrow = es.enter_context(tc.tile_pool(name="wrow", bufs=2))
        wmlp = es.enter_context(tc.tile_pool(name="wmlp", bufs=6))
        wdp = es.enter_context(tc.tile_pool(name="wdp", bufs=8))
        sqp = es.enter_context(tc.tile_pool(name="sq", bufs=4))
        smallp = es.enter_context(tc.tile_pool(name="small", bufs=4))
        probsp = es.enter_context(tc.tile_pool(name="probs", bufs=4))
        ptp = es.enter_context(tc.tile_pool(name="ptp", bufs=3))
        attnp = es.enter_context(tc.tile_pool(name="attn", bufs=2))
        mtilep = es.enter_context(tc.tile_pool(name="mtile", bufs=4))
        dchp = es.enter_context(tc.tile_pool(name="dch", bufs=2))
        arpool = es.enter_context(tc.tile_pool(name="arp", bufs=3))
        outp = es.enter_context(tc.tile_pool(name="outp", bufs=3))
        invp = es.enter_context(tc.tile_pool(name="invp", bufs=3))
        pp = es.enter_context(tc.tile_pool(name="psum", bufs=8, space="PSUM"))
        dram = es.enter_context(tc.tile_pool(name="dram", bufs=1, space="DRAM"))
        if True:
            # ---------------- collective warmup (first!) ----------------
            warm_src = smallp.tile([128, 16], BF16)
            nc.vector.memset(warm_src[:], 0.0)
            warm_in = dram.tile([128, 16], BF16)
            warm_out = dram.tile([128, 16], BF16, addr_space="Shared")
            nc.sync.dma_start(out=warm_in[:], in_=warm_src[:])
            nc.gpsimd.collective_compute(
                "AllReduce", ALU.add, replica_groups=RG,
                ins=[warm_in.opt()], outs=[warm_out.opt()])
            warm_sb = smallp.tile([1, 16], BF16)
            nc.sync.dma_start(out=warm_sb[:], in_=warm_out[0:1, :])

            # ---------------- persistent tiles ----------------
            X = {}    # f32 master residual stream
            XB = {}   # bf16 copy fed to matmuls
            for b in range(B):
                X[b] = persist.tile([128, NDC * S], F32, name=f"Xb{b}",
                                    tag=f"Xb{b}")
                XB[b] = persist.tile([128, NDC * S], BF16, name=f"XBb{b}",
                                     tag=f"XBb{b}")
            ident = persist.tile([128, 128], BF16)
            masks.make_identity(nc, ident[:])
            ones_col = persist.tile([128, 1], BF16)
            nc.vector.memset(ones_col[:], 1.0)
            ones_row = persist.tile([1, 128], F32)
            nc.vector.memset(ones_row[:], 1.0)
            eps_t = persist.tile([1, 1], F32)
            nc.vector.memset(eps_t[:], EPS)
            sin_sb = persist.tile([64, B * S], F32)
            cos_sb = persist.tile([64, B * S], F32)
            smask_sb = persist.tile([S, B * KA], BF16)
            cmask_sb = persist.tile([S, B * KT], BF16)
            fnw_sb = persist.tile([128, NDC], F32)

            for b in range(B):
                nc.sync.dma_start(
                    out=X[b][:].rearrange("p (c s) -> p c s", s=S),
                    in_=x_in[:, b * S:(b + 1) * S].rearrange(
                        "(c p) s -> p c s", p=128))
                for g in range(4):
                    nc.vector.tensor_copy(
                        XB[b][:, g * 4 * S:(g + 1) * 4 * S],
                        X[b][:, g * 4 * S:(g + 1) * 4 * S])
            nc.sync.dma_start(out=sin_sb[:], in_=sinT[:])
            nc.sync.dma_start(out=cos_sb[:], in_=cosT[:])
            nc.sync.dma_start(out=smask_sb[:], in_=smask[:])
            nc.sync.dma_start(out=cmask_sb[:], in_=cmask[:])
            nc.sync.dma_start(out=fnw_sb[:], in_=fnw[:])

            # AR bounce buffers, one pair per (reduction point, batch)
            ar_bufs = {}
            for k in range(3 * L):
                for b in range(B):
                    ar_bufs[(k, b)] = (
                        dram.tile([D, S], BF16, name=f"arin{k}_{b}", tag=f"arin{k}_{b}"),
                        dram.tile([D, S], BF16, name=f"arout{k}_{b}", tag=f"arout{k}_{b}",
                                  addr_space="Shared"),
                    )

            # ---------------- helpers ----------------
            def residual(slot, b, last=False):
                """Read back AR output for (slot, b); update XB (bf16, hot
                path for the next GEMMs) first, then the f32 master X."""
                arout = ar_bufs[(slot, b)][1]
                art = arpool.tile([128, NDC * S], BF16, tag="ar", bufs=1)
                nc.sync.dma_start(
                    out=art[:].rearrange("p (c s) -> p c s", s=S),
                    in_=arout[:].rearrange("(c p) s -> p c s", p=128))
                if last:
                    for g in range(4):
                        sl = slice(g * 4 * S, (g + 1) * 4 * S)
                        nc.vector.tensor_add(X[b][:, sl], X[b][:, sl], art[:, sl])
                    return
                for g in range(4):
                    sl = slice(g * 4 * S, (g + 1) * 4 * S)
                    nc.vector.tensor_add(XB[b][:, sl], X[b][:, sl], art[:, sl])
                for g in range(4):
                    sl = slice(g * 4 * S, (g + 1) * 4 * S)
                    nc.vector.tensor_add(X[b][:, sl], X[b][:, sl], art[:, sl])

            def stats_invb(b):
                """[128, S] broadcast tile of 1/rms(X_new[:, b]) from XB."""
                nw = pp.tile([1, 4 * S], F32, tag="psum", name=f"nw{nc.next_id()}")
                for g in range(4):
                    sq = sqp.tile([128, 4 * S], BF16, tag="sq", bufs=1)
                    nc.scalar.activation(sq[:], XB[b][:, g * 4 * S:(g + 1) * 4 * S],
                                         AF.Square)
                    nc.tensor.matmul(nw[:], ones_col[:], sq[:],
                                     start=(g == 0), stop=(g == 3))
                nwsb = smallp.tile([1, 4 * S], F32, tag="nwsb", bufs=1)
                nc.vector.tensor_copy(nwsb[:], nw[:])
                t0 = smallp.tile([1, S], F32, tag="nfold", bufs=3)
                t1 = smallp.tile([1, S], F32, tag="nfold", bufs=3)
                nc.vector.tensor_add(t0[:], nwsb[:, 0:S], nwsb[:, S:2 * S])
                nc.vector.tensor_add(t1[:], nwsb[:, 2 * S:3 * S], nwsb[:, 3 * S:4 * S])
                nstat = smallp.tile([1, S], F32, tag="nfold", bufs=3)
                nc.vector.tensor_add(nstat[:], t0[:], t1[:])
                sd = smallp.tile([1, S], F32, tag="sd")
                nc.scalar.activation(sd[:], nstat[:], AF.Sqrt,
                                     bias=eps_t[:], scale=1.0 / D)
                inv = smallp.tile([1, S], F32, tag="inv")
                nc.vector.reciprocal(inv[:], sd[:])
                invps = pp.tile([128, S], F32, tag="psum", name=f"inb{nc.next_id()}")
                nc.tensor.matmul(invps[:], ones_row[:], inv[:], start=True, stop=True)
                invb = invp.tile([128, S], F32, tag="invb", bufs=2)
                nc.vector.tensor_copy(invb[:], invps[:])
                return invb

            def scaled_sincos(b, invb):
                sn = smallp.tile([64, S], F32, tag="ssc", bufs=2)
                cs = smallp.tile([64, S], F32, tag="ssc", bufs=2)
                nc.vector.tensor_mul(sn[:], sin_sb[:, b * S:(b + 1) * S],
                                     invb[0:64, :])
                nc.vector.tensor_mul(cs[:], cos_sb[:, b * S:(b + 1) * S],
                                     invb[0:64, :])
                return sn, cs

            def rope_into(dst, dst_col, src_ap, sn, cs):
                t1 = smallp.tile([64, S], F32, tag="ropet", bufs=4)
                t2 = smallp.tile([64, S], F32, tag="ropet", bufs=4)
                x1 = src_ap[0:64, 0:S]
                x2 = src_ap[64:128, 0:S]
                nc.vector.tensor_mul(t1[:], cs[:], x1)
                nc.vector.tensor_mul(t2[:], sn[:], x2)
                nc.vector.tensor_sub(dst[0:64, dst_col:dst_col + S], t1[:], t2[:])
                t3 = smallp.tile([64, S], F32, tag="ropet", bufs=4)
                t4 = smallp.tile([64, S], F32, tag="ropet", bufs=4)
                nc.vector.tensor_mul(t3[:], cs[:], x2)
                nc.vector.tensor_mul(t4[:], sn[:], x1)
                nc.vector.tensor_add(dst[64:128, dst_col:dst_col + S], t3[:], t4[:])

            def ar_issue(slot, b, delta_sb):
                arin, arout = ar_bufs[(slot, b)]
                nc.sync.dma_start(
                    out=arin[:].rearrange("(c p) s -> p c s", p=128),
                    in_=delta_sb[:].rearrange("p (c s) -> p c s", s=S))
                nc.gpsimd.collective_compute(
                    "AllReduce", ALU.add, replica_groups=RG,
                    ins=[arin.opt()], outs=[arout.opt()])

            def second_proj(wts, act_sb, n_e):
                """delta[d, s] = sum_e W[e, d] act[e, s]; act_sb [128, n_e*S]
                cols h*S+s. wts: list of n_e weight tiles [128(e), D]."""
                delta_sb = dchp.tile([128, NDC * S], BF16, tag="dsb", bufs=2)
                for dc in range(NDC):
                    pd = pp.tile([128, S], F32, tag="psum", name=f"pd{nc.next_id()}")
                    for ec in range(n_e):
                        nc.tensor.matmul(
                            pd[:], wts[ec][:, dc * 128:(dc + 1) * 128],
                            act_sb[:, ec * S:(ec + 1) * S],
                            start=(ec == 0), stop=(ec == n_e - 1))
                    nc.scalar.activation(
                        delta_sb[:, dc * S:(dc + 1) * S], pd[:], AF.Copy)
                return delta_sb

            def load_layer_caches(l):
                st = {}
                for b in range(B):
                    kt = cachep.tile([128, KA], BF16, tag=f"kT{b}", bufs=1)
                    nc.sync.dma_start(out=kt[:], in_=kT[l, b])
                    st[("kT", b)] = kt
                    vt = cachep.tile([128, nkc * 128], BF16, tag=f"vTb{b}", bufs=1)
                    nc.sync.dma_start(
                        out=vt[:].rearrange("p (c f) -> p c f", f=128),
                        in_=vT[l, b].rearrange("(c p) f -> p c f", p=128))
                    st[("vTb", b)] = vt
                    for h in range(CH):
                        ck = cachep.tile([128, KT], BF16, tag=f"ckT{b}_{h}", bufs=1)
                        nc.sync.dma_start(out=ck[:], in_=ckT[l, b, h])
                        st[("ckT", b, h)] = ck
                        cv = cachep.tile([128, ntck * 128], BF16, tag=f"cvTb{b}_{h}", bufs=1)
                        nc.sync.dma_start(
                            out=cv[:].rearrange("p (c f) -> p c f", f=128),
                            in_=cvT[l, b, h].rearrange("(c p) f -> p c f", p=128))
                        st[("cvTb", b, h)] = cv
                return st

            def softmax_rows(ps_list, widths, p, mask_sb, mask_off):
                """exp+normalize a row-softmax split over groups.
                ps_list[g]: psum [S, widths[g]]; p: sbuf [S, sum(widths)] out."""
                dparts = []
                col = 0
                for g, ps in enumerate(ps_list):
                    wdt = widths[g]
                    nc.vector.tensor_add(
                        ps[:], ps[:],
                        mask_sb[:, mask_off + col:mask_off + col + wdt])
                    dp_ = smallp.tile([S, 1], F32, tag="denom", bufs=8)
                    nc.scalar.activation(p[:, col:col + wdt], ps[:], AF.Exp,
                                         accum_out=dp_[:])
                    dparts.append(dp_)
                    col += wdt
                denom = dparts[0]
                for dp_ in dparts[1:]:
                    dnew = smallp.tile([S, 1], F32, tag="denom", bufs=8)
                    nc.vector.tensor_add(dnew[:], denom[:], dp_[:])
                    denom = dnew
                invd = smallp.tile([S, 1], F32, tag="invd", bufs=4)
                nc.vector.reciprocal(invd[:], denom[:])
                nc.vector.tensor_scalar_mul(p[:], p[:], invd[:])

            def sa_block(l, b, st):
                invb = stats_invb(b)
                if b == 0:
                    qw_sb = wbig.tile([128, NDC * QH * HD], BF16, tag="wq", bufs=2)
                    nc.scalar.dma_start(
                        out=qw_sb[:].rearrange("p (c e) -> p c e", e=QH * HD),
                        in_=qw[l].rearrange("(c p) e -> p c e", p=128))
                    kv_sb = wbig.tile([128, NDC * 2 * HD], BF16, tag="wkv", bufs=1)
                    nc.scalar.dma_start(
                        out=kv_sb[:].rearrange("p (c e) -> p c e", e=2 * HD),
                        in_=kvw[l].rearrange("(c p) e -> p c e", p=128))
                    st["qw"], st["kv"] = qw_sb, kv_sb
                qw_sb, kv_sb = st["qw"], st["kv"]

                pq0 = pp.tile([128, S], F32, tag="psum", name=f"pq0{nc.next_id()}")
                pq1 = pp.tile([128, S], F32, tag="psum", name=f"pq1{nc.next_id()}")
                pk = pp.tile([128, S], F32, tag="psum", name=f"pk{nc.next_id()}")
                pv = pp.tile([128, S], F32, tag="psum", name=f"pv{nc.next_id()}")
                for i in range(NDC):
                    stt, spp = (i == 0), (i == NDC - 1)
                    qo, ko = i * QH * HD, i * 2 * HD
                    hsl = XB[b][:, i * S:(i + 1) * S]
                    nc.tensor.matmul(pq0[:], qw_sb[:, qo:qo + 128], hsl, start=stt, stop=spp)
                    nc.tensor.matmul(pq1[:], qw_sb[:, qo + 128:qo + 256], hsl, start=stt, stop=spp)
                    nc.tensor.matmul(pk[:], kv_sb[:, ko:ko + 128], hsl, start=stt, stop=spp)
                    nc.tensor.matmul(pv[:], kv_sb[:, ko + 128:ko + 256], hsl, start=stt, stop=spp)

                sn, cs = scaled_sincos(b, invb)
                q_roped = probsp.tile([128, QH * S], BF16, tag="qrope", bufs=2)
                rope_into(q_roped, 0, pq0[:], sn, cs)
                rope_into(q_roped, S, pq1[:], sn, cs)
                rope_into(st[("kT", b)], w, pk[:], sn, cs)

                vsb = probsp.tile([128, S], BF16, tag="vsb", bufs=2)
                nc.vector.tensor_mul(vsb[:], pv[:], invb[:])
                pvt = pp.tile([128, 128], BF16, tag="psum", name=f"pvt{nc.next_id()}")
                nc.tensor.transpose(pvt[:], vsb[:], ident[:])
                r, c0 = w % 128, w // 128
                vtb = st[("vTb", b)]
                if r == 0:
                    nc.vector.tensor_copy(vtb[:, c0 * 128:(c0 + 1) * 128], pvt[:])
                else:
                    nc.vector.tensor_copy(vtb[r:128, c0 * 128:(c0 + 1) * 128],
                                          pvt[0:128 - r, :])
                    nc.vector.tensor_copy(vtb[0:r, (c0 + 1) * 128:(c0 + 2) * 128],
                                          pvt[128 - r:128, :])

                p_tiles = {}
                for h in range(QH):
                    p = probsp.tile([S, KA], BF16, tag="p", bufs=2)
                    ps_list, widths = [], []
                    for g0 in range(0, nkc, 4):
                        wdt = min(4, nkc - g0) * 128
                        ps = pp.tile([S, wdt], F32, tag="psum", name=f"ps{nc.next_id()}")
                        nc.tensor.matmul(
                            ps[:], q_roped[:, h * S:(h + 1) * S],
                            st[("kT", b)][:, g0 * 128:g0 * 128 + wdt],
                            start=True, stop=True)
                        ps_list.append(ps)
                        widths.append(wdt)
                    softmax_rows(ps_list, widths, p, smask_sb, b * KA)
                    p_tiles[h] = p

                pattn = pp.tile([128, QH * S], F32, tag="psum", name=f"pat{nc.next_id()}")
                for j in range(nkc):
                    pT = ptp.tile([128, QH * S], BF16, tag="pT", bufs=2)
                    for h in range(QH):
                        ptps = pp.tile([S, 128], BF16, tag="psum", name=f"ptp{nc.next_id()}")
                        nc.tensor.transpose(
                            ptps[:], p_tiles[h][:, j * 128:(j + 1) * 128], ident[:])
                        nc.vector.tensor_copy(pT[:, h * S:(h + 1) * S], ptps[:])
                    nc.tensor.matmul(
                        pattn[:], st[("vTb", b)][:, j * 128:(j + 1) * 128], pT[:],
                        start=(j == 0), stop=(j == nkc - 1))
                attn_sb = attnp.tile([128, QH * S], BF16, tag="attn")
                nc.scalar.activation(attn_sb[:], pattn[:], AF.Copy)

                if b == 0:
                    wts = []
                    for ec in range(QH):
                        wt = wrow.tile([128, D], BF16, tag="wrow", bufs=2)
                        nc.scalar.dma_start(out=wt[:], in_=ow[l, ec * 128:(ec + 1) * 128, :])
                        wts.append(wt)
                    st["ow"] = wts
                return second_proj(st["ow"], attn_sb, QH)

            def ca_block(l, b, st):
                invb = stats_invb(b)
                if b == 0:
                    cq_sb = wbig.tile([128, NDC * CH * HDC], BF16, tag="wq", bufs=2)
                    nc.scalar.dma_start(
                        out=cq_sb[:].rearrange("p (c e) -> p c e", e=CH * HDC),
                        in_=cqw[l].rearrange("(c p) e -> p c e", p=128))
                    st["cq"] = cq_sb
                cq_sb = st["cq"]
                pcq0 = pp.tile([128, S], F32, tag="psum", name=f"pcq0{nc.next_id()}")
                pcq1 = pp.tile([128, S], F32, tag="psum", name=f"pcq1{nc.next_id()}")
                for i in range(NDC):
                    stt, spp = (i == 0), (i == NDC - 1)
                    qo = i * CH * HDC
                    hsl = XB[b][:, i * S:(i + 1) * S]
                    nc.tensor.matmul(pcq0[:], cq_sb[:, qo:qo + 128], hsl, start=stt, stop=spp)
                    nc.tensor.matmul(pcq1[:], cq_sb[:, qo + 128:qo + 256], hsl, start=stt, stop=spp)
                sn, cs = scaled_sincos(b, invb)
                cq_roped = probsp.tile([128, CH * S], BF16, tag="qrope", bufs=2)
                rope_into(cq_roped, 0, pcq0[:], sn, cs)
                rope_into(cq_roped, S, pcq1[:], sn, cs)

                cattn_sb = attnp.tile([128, CH * S], BF16, tag="attn")
                for h in range(CH):
                    ps = pp.tile([S, KT], F32, tag="psum", name=f"cps{nc.next_id()}")
                    nc.tensor.matmul(
                        ps[:], cq_roped[:, h * S:(h + 1) * S],
                        st[("ckT", b, h)][:], start=True, stop=True)
                    p = probsp.tile([S, KT], BF16, tag="cp", bufs=2)
                    softmax_rows([ps], [KT], p, cmask_sb, b * KT)
                    pcat = pp.tile([128, S], F32, tag="psum", name=f"pca{nc.next_id()}")
                    for j in range(ntck):
                        pT = ptp.tile([S, 128], BF16, tag="cpT", bufs=2)
                        ptps = pp.tile([S, 128], BF16, tag="psum", name=f"ptc{nc.next_id()}")
                        nc.tensor.transpose(
                            ptps[:], p[:, j * 128:(j + 1) * 128], ident[:])
                        nc.vector.tensor_copy(pT[:], ptps[:])
                        nc.tensor.matmul(
                            pcat[:], st[("cvTb", b, h)][:, j * 128:(j + 1) * 128], pT[:],
                            start=(j == 0), stop=(j == ntck - 1))
                    nc.scalar.activation(cattn_sb[:, h * S:(h + 1) * S],
                                         pcat[:], AF.Copy)

                if b == 0:
                    wts = []
                    for ec in range(CH):
                        wt = wrow.tile([128, D], BF16, tag="wrow", bufs=2)
                        nc.scalar.dma_start(out=wt[:], in_=cow[l, ec * 128:(ec + 1) * 128, :])
                        wts.append(wt)
                    st["cow"] = wts
                return second_proj(st["cow"], cattn_sb, CH)

            def mlp_block_b(l, b, st):
                """MLP for one batch; weights loaded at b==0 stay resident."""
                invb = stats_invb(b)
                WW = 2
                if b == 0:
                    gts, uts = [], []
                    for wv in range(NFC // WW):
                        gt = wmlp.tile([128, NDC * WW * 128], BF16,
                                       tag=f"wg{wv}", bufs=1)
                        nc.scalar.dma_start(
                            out=gt[:].rearrange("p (c f) -> p c f", f=WW * 128),
                            in_=wgw[l, :, wv * WW * 128:(wv + 1) * WW * 128].rearrange(
                                "(c p) f -> p c f", p=128))
                        ut = wmlp.tile([128, NDC * WW * 128], BF16,
                                       tag=f"wu{wv}", bufs=1)
                        nc.scalar.dma_start(
                            out=ut[:].rearrange("p (c f) -> p c f", f=WW * 128),
                            in_=wuw[l, :, wv * WW * 128:(wv + 1) * WW * 128].rearrange(
                                "(c p) f -> p c f", p=128))
                        gts.append(gt)
                        uts.append(ut)
                    st["wg"], st["wu"] = gts, uts
                    wd_sb = []
                    for dhalf in range(2):
                        for fg in range(2):
                            t = wdp.tile([128, 4 * (D // 2)], BF16,
                                         tag=f"wd{dhalf}_{fg}", bufs=1)
                            nc.scalar.dma_start(
                                out=t[:].rearrange("p (c f) -> p c f", f=D // 2),
                                in_=wdw[l, fg * 512:(fg + 1) * 512,
                                        dhalf * (D // 2):(dhalf + 1) * (D // 2)].rearrange(
                                    "(c p) f -> p c f", p=128))
                            wd_sb.append(t)
                    st["wd"] = wd_sb
                gts, uts, wd_sb = st["wg"], st["wu"], st["wd"]

                m_tiles = []
                for wv in range(NFC // WW):
                    gt, ut = gts[wv], uts[wv]
                    for k in range(WW):
                        pg = pp.tile([128, S], F32, tag="psum", name=f"pg{nc.next_id()}")
                        pu = pp.tile([128, S], F32, tag="psum", name=f"pu{nc.next_id()}")
                        for i in range(NDC):
                            stt, spp = (i == 0), (i == NDC - 1)
                            co = i * WW * 128 + k * 128
                            hsl = XB[b][:, i * S:(i + 1) * S]
                            nc.tensor.matmul(pg[:], gt[:, co:co + 128],
                                             hsl, start=stt, stop=spp)
                            nc.tensor.matmul(pu[:], ut[:, co:co + 128],
                                             hsl, start=stt, stop=spp)
                        sgs = mtilep.tile([128, S], BF16, tag="sgs", bufs=2)
                        nc.vector.tensor_mul(sgs[:], pg[:], invb[:])
                        sg = mtilep.tile([128, S], BF16, tag="sg")
                        nc.scalar.activation(sg[:], sgs[:], AF.Silu)
                        ub = mtilep.tile([128, S], BF16, tag="ub")
                        nc.vector.tensor_mul(ub[:], pu[:], invb[:])
                        m = mtilep.tile([128, S], BF16, tag="m", bufs=12)
                        nc.vector.tensor_mul(m[:], sg[:], ub[:])
                        m_tiles.append(m)

                delta = dchp.tile([128, NDC * S], BF16, tag="dsb", bufs=2,
                                  name=f"dmlp{l}_{b}")
                for dhalf in range(2):
                    for dc8 in range(NDC // 2):
                        dc = dhalf * (NDC // 2) + dc8
                        pd = pp.tile([128, S], F32, tag="psum", name=f"pdm{nc.next_id()}")
                        for fc in range(NFC):
                            t = wd_sb[dhalf * 2 + fc // 4]
                            co = (fc % 4) * (D // 2) + dc8 * 128
                            nc.tensor.matmul(
                                pd[:], t[:, co:co + 128],
                                m_tiles[fc][:], start=(fc == 0), stop=(fc == NFC - 1))
                        nc.scalar.activation(
                            delta[:, dc * S:(dc + 1) * S], pd[:], AF.Copy)
                return delta

            # ================= layers (batch-staggered pipeline) =========
            for l in range(L):
                st = load_layer_caches(l)
                # SA
                for b in range(B):
                    if l > 0:
                        residual(3 * (l - 1) + 2, b)
                    d = sa_block(l, b, st)
                    ar_issue(3 * l + 0, b, d)
                # CA
                for b in range(B):
                    residual(3 * l + 0, b)
                    d = ca_block(l, b, st)
                    ar_issue(3 * l + 1, b, d)
                # MLP (batch-sequential: AR of b0 hides under b1 compute)
                for b in range(B):
                    residual(3 * l + 1, b)
                    d = mlp_block_b(l, b, st)
                    ar_issue(3 * l + 2, b, d)

            # ================ final norm + output ================
            nc.vector.tensor_scalar_mul(warm_sb[:], warm_sb[:], 0.0)
            for b in range(B):
                residual(3 * (L - 1) + 2, b, last=True)
                nw = pp.tile([1, 4 * S], F32, tag="psum", name=f"nsf{b}")
                for g in range(4):
                    sq = sqp.tile([128, 4 * S], BF16, tag="sq", bufs=1)
                    nc.scalar.activation(sq[:], X[b][:, g * 4 * S:(g + 1) * 4 * S],
                                         AF.Square)
                    nc.tensor.matmul(nw[:], ones_col[:], sq[:],
                                     start=(g == 0), stop=(g == 3))
                nwsb = smallp.tile([1, 4 * S], F32, tag="nwsb", bufs=1)
                nc.vector.tensor_copy(nwsb[:], nw[:])
                t0 = smallp.tile([1, S], F32, tag="nfold", bufs=3)
                t1 = smallp.tile([1, S], F32, tag="nfold", bufs=3)
                nc.vector.tensor_add(t0[:], nwsb[:, 0:S], nwsb[:, S:2 * S])
                nc.vector.tensor_add(t1[:], nwsb[:, 2 * S:3 * S], nwsb[:, 3 * S:4 * S])
                nstat = smallp.tile([1, S], F32, tag="nfold", bufs=3)
                nc.vector.tensor_add(nstat[:], t0[:], t1[:])
                sd = smallp.tile([1, S], F32, tag="sd")
                nc.scalar.activation(sd[:], nstat[:], AF.Sqrt,
                                     bias=eps_t[:], scale=1.0 / D)
                inv = smallp.tile([1, S], F32, tag="inv")
                nc.vector.reciprocal(inv[:], sd[:])
                invb = pp.tile([128, S], F32, tag="psum", name=f"inf{b}")
                nc.tensor.matmul(invb[:], ones_row[:], inv[:], start=True, stop=True)
                for i in range(NDC):
                    t = outp.tile([128, S], F32, tag="outf", bufs=1)
                    nc.vector.tensor_mul(t[:], X[b][:, i * S:(i + 1) * S], invb[:])
                    o = outp.tile([128, S], F32, tag="outo", bufs=1)
                    nc.scalar.activation(o[:], t[:], AF.Copy, scale=fnw_sb[:, i:i + 1])
                    if i == 0 and b == 0:
                        nc.vector.tensor_add(o[0:1, 0:16], o[0:1, 0:16], warm_sb[:])
                    nc.sync.dma_start(
                        out=out_ext[i * 128:(i + 1) * 128, b * S:(b + 1) * S],
                        in_=o[:])

    nc.finalize()
    return nc


# ---------------------------------------------------------------- host prep
def _prep_in_maps(inputs):
    f32 = np.float32
    x = inputs["x"].astype(f32)                      # (B, D, 1, S)
    positions = inputs["positions"]
    w = int(np.asarray(inputs["kv_write_index"]).reshape(-1)[0])
    self_attn_mask = inputs["self_attn_mask"].astype(f32)  # (B,1,S,A)
    enc_len = np.asarray(inputs["encoder_lengths"]).reshape(B)

    sa_n = inputs["sa_norm_w"].astype(f32)[:, :, None]     # (L, D, 1)
    ca_n = inputs["ca_norm_w"].astype(f32)[:, :, None]
    mlp_n = inputs["mlp_norm_w"].astype(f32)[:, :, None]
    scale = 1.0 / np.sqrt(HD).astype(f32)
    cscale = 1.0 / np.sqrt(HDC).astype(f32)

    qw = (inputs["q_w"] * sa_n * scale).astype(BF)         # (L, D, HQ*HD)
    kw = (inputs["k_w"] * sa_n).astype(BF)
    vw = (inputs["v_w"] * sa_n).astype(BF)
    ow = inputs["o_w"].astype(BF)                          # (L, HQ*HD, D)
    cqw = (inputs["cq_w"] * ca_n * cscale).astype(BF)
    cow = inputs["co_w"].astype(BF)
    wgw = (inputs["wg_w"] * mlp_n).astype(BF)
    wuw = (inputs["wu_w"] * mlp_n).astype(BF)
    wdw = inputs["wd_w"].astype(BF)

    # --- masked-chunk classification -------------------------------------
    # self-attn: chunk c of the A axis is skippable if fully masked for all
    # queries in both batches; visible chunks form a prefix for causal masks.
    m = self_attn_mask[:, 0]                               # (B, S, A)
    ch = m.reshape(B, S, A // 128, 128)
    full_masked = (ch < -1e3).all(axis=(0, 1, 3))          # (A//128,)
    keep = np.nonzero(~full_masked)[0]
    if len(keep) == 0:
        nkc = A // 128
        m = m + 3e4
    else:
        nkc = int(keep[-1]) + 1
    # ensure the KV write region is covered
    nkc = max(nkc, (w + S + 127) // 128)
    nkc = min(nkc, A // 128)
    KA = nkc * 128

    # cross-attn: valid keys are t < enc_len (prefix); fully-masked batch
    # falls back to all chunks with a shifted (all-zero) mask == uniform.
    if (enc_len <= 0).any():
        ntck = T // 128
    else:
        ntck = int((enc_len.max() + 127) // 128)
    KT = ntck * 128

    k_cache = inputs["k_cache"].reshape(L, B, HKV, A, HD)
    v_cache = inputs["v_cache"].reshape(L, B, HKV, HD, A)
    ck = inputs["ck_cache"].reshape(L, B, HC, T, HDC)
    cv = inputs["cv_cache"].reshape(L, B, HC, HDC, T)
    kTf = np.ascontiguousarray(
        k_cache.transpose(0, 1, 2, 4, 3)[:, :, :, :, :KA]).astype(BF)
    vTf = np.ascontiguousarray(
        v_cache.transpose(0, 1, 2, 4, 3)[:, :, :, :KA, :]).astype(BF)
    ckTf = np.ascontiguousarray(
        ck.transpose(0, 1, 2, 4, 3)[:, :, :, :, :KT]).astype(BF)
    cvTf = np.ascontiguousarray(
        cv.transpose(0, 1, 2, 4, 3)[:, :, :KT, :]).astype(BF)

    inv_freq = 1.0 / (10000.0 ** (np.arange(0, HD, 2, dtype=f32) / HD))
    ang = positions.astype(f32)[:, None, :] * inv_freq[None, :, None]   # (B,64,S)
    sinT = np.ascontiguousarray(np.sin(ang).transpose(1, 0, 2).reshape(64, B * S)).astype(f32)
    cosT = np.ascontiguousarray(np.cos(ang).transpose(1, 0, 2).reshape(64, B * S)).astype(f32)

    smask = np.ascontiguousarray(
        m[:, :, :KA].transpose(1, 0, 2).reshape(S, B * KA)).astype(BF)
    t_idx = np.arange(KT)
    cm = np.where(t_idx[None, :] < enc_len[:, None], 0.0, NEG).astype(f32)
    for b in range(B):
        if enc_len[b] <= 0:
            cm[b] += 3e4
    cmask = np.ascontiguousarray(
        np.broadcast_to(cm.reshape(1, B * KT), (S, B * KT))).astype(BF)

    x_in = np.ascontiguousarray(
        x[:, :, 0, :].transpose(1, 0, 2).reshape(D, B * S)).astype(f32)
    fnw = np.ascontiguousarray(
        inputs["final_norm_w"].astype(f32).reshape(NDC, 128).T)

    in_maps = []
    for c in range(NCORES):
        qh = slice(2 * c * HD, (2 * c + 2) * HD)
        kvh = c // 2
        ffs = slice(c * FFS, (c + 1) * FFS)
        in_maps.append({
            "x_in": x_in,
            "qw": np.ascontiguousarray(qw[:, :, qh]),
            "kvw": np.ascontiguousarray(np.concatenate(
                [kw[:, :, kvh * HD:(kvh + 1) * HD],
                 vw[:, :, kvh * HD:(kvh + 1) * HD]], axis=2)),
            "ow": np.ascontiguousarray(ow[:, qh, :]),
            "cqw": np.ascontiguousarray(cqw[:, :, qh]),
            "cow": np.ascontiguousarray(cow[:, qh, :]),
            "wgw": np.ascontiguousarray(wgw[:, :, ffs]),
            "wuw": np.ascontiguousarray(wuw[:, :, ffs]),
            "wdw": np.ascontiguousarray(wdw[:, ffs, :]),
            "kT": np.ascontiguousarray(kTf[:, :, kvh]),
            "vT": np.ascontiguousarray(vTf[:, :, kvh]),
            "ckT": np.ascontiguousarray(ckTf[:, :, 2 * c:2 * c + 2]),
            "cvT": np.ascontiguousarray(cvTf[:, :, 2 * c:2 * c + 2]),
            "sinT": sinT, "cosT": cosT,
            "smask": smask, "cmask": cmask,
            "fnw": fnw,
        })
    return in_maps, w, nkc, ntck


def kernel(**inputs):
    global _exec_time_ns
    in_maps, w, nkc, ntck = _prep_in_maps(inputs)
    nc = build_graph(w, nkc, ntck)
    trace = bool(int(os.environ.get("BASS_KERNEL_TRACE", "0")))
    res = run_bass_kernel_spmd(nc, in_maps, list(range(NCORES)), trace=trace)
    _exec_time_ns = res.exec_time_ns
    out = np.asarray(res.results[0]["out"])          # [D, B*S] f32
    out = out.reshape(D, B, S).transpose(1, 0, 2)[:, :, None, :]
    return np.ascontiguousarray(out.astype(np.float32))


# revision 15
# speedup vs baseline: 1.0851x; 1.0851x over previous
"""Trainium2 Bass kernel for nn_ANEDecoder (Dia-style ANE decoder, 2 layers).

Sharding: tensor-parallel across 8 cores — 2 query heads + their (shared)
KV head per core for self-attn, 2 cross heads per core, FF/8 = 1024 MLP
hidden per core, all-reduce after o_proj / co_proj / wd.

v3 over v2:
- Post-scale RMSNorm: blocks matmul the *un-normalized* residual (XB, bf16)
  immediately after the all-reduce lands; the 1/rms factor is applied to the
  (much smaller) projection outputs, so the stats chain overlaps the GEMMs
  instead of serializing in front of them.
- Masked-chunk skipping: self-attention only touches the ceil((w+S)/128)
  cache chunks the causal mask can reach (3 of 12 for w=256); cross-attn
  only ceil(enc_len/128) chunks. Fully-masked chunks contribute exp(-3e4)=0
  exactly, so skipping is lossless.
- AR readback on the sync queue (gpsimd queue carries only collectives),
  residual add writes the bf16 GEMM input directly (f32 master copy updated
  off-path on gpsimd).
- Warmup collective issued as the first instruction so the one-time CC
  bring-up overlaps the prologue DMA.
"""
import os
import sys
import functools
from contextlib import ExitStack

sys.path.insert(0, "/opt/trn_rl_repo")

import numpy as np
import ml_dtypes

import concourse.bass as bass
import concourse.bacc as bacc
import concourse.mybir as mybir
import concourse.tile as tile
import concourse.masks as masks
from concourse.bass_utils import run_bass_kernel_spmd

BF = ml_dtypes.bfloat16
F32 = mybir.dt.float32
BF16 = mybir.dt.bfloat16
AF = mybir.ActivationFunctionType
ALU = mybir.AluOpType

# dims
B, D, S = 2, 2048, 128
A, T = 1536, 512
HQ, HKV, HD = 16, 4, 128
HC, HDC = 16, 128
FF, L = 8192, 2
EPS = 1e-5
NEG = -3e4

NCORES = 8
NDC = D // 128          # 16 d-chunks
QH = HQ // NCORES       # 2 query heads per core
CH = HC // NCORES       # 2 cross heads per core
FFS = FF // NCORES      # 1024 ff per core
NFC = FFS // 128        # 8 ff chunks

_exec_time_ns = None


def last_exec_time_ns():
    return _exec_time_ns


# ---------------------------------------------------------------- builder
@functools.lru_cache(maxsize=4)
def build_graph(w: int, nkc: int, ntck: int):
    KA = nkc * 128          # kept self-attn cache columns
    KT = ntck * 128         # kept cross-attn cache columns
    nc = bacc.Bacc()

    def par(name, shape, dt):
        return nc.declare_dram_parameter(name, list(shape), dt, isOutput=False)

    x_in = par("x_in", (D, B * S), F32)
    qw = par("qw", (L, D, QH * HD), BF16)
    kvw = par("kvw", (L, D, 2 * HD), BF16)
    ow = par("ow", (L, QH * HD, D), BF16)
    cqw = par("cqw", (L, D, CH * HDC), BF16)
    cow = par("cow", (L, CH * HDC, D), BF16)
    wgw = par("wgw", (L, D, FFS), BF16)
    wuw = par("wuw", (L, D, FFS), BF16)
    wdw = par("wdw", (L, FFS, D), BF16)
    kT = par("kT", (L, B, HD, KA), BF16)
    vT = par("vT", (L, B, KA, HD), BF16)
    ckT = par("ckT", (L, B, CH, HDC, KT), BF16)
    cvT = par("cvT", (L, B, CH, KT, HDC), BF16)
    sinT = par("sinT", (HD // 2, B * S), F32)
    cosT = par("cosT", (HD // 2, B * S), F32)
    smask = par("smask", (S, B * KA), BF16)
    cmask = par("cmask", (S, B * KT), BF16)
    fnw = par("fnw", (128, NDC), F32)
    out_ext = nc.declare_dram_parameter("out", [D, B * S], F32, isOutput=True)

    RG = [list(range(NCORES))]

    with tile.TileContext(nc) as tc, ExitStack() as es:
        persist = es.enter_context(tc.tile_pool(name="persist", bufs=1))
        cachep = es.enter_context(tc.tile_pool(name="cache", bufs=2))
        wbig = es.enter_context(tc.tile_pool(name="wbig", bufs=2))
        wrow = es.enter_context(tc.tile_pool(name="wrow", bufs=2))
        wmlp = es.enter_context(tc.tile_pool(name="wmlp", bufs=6))
        wdp = es.enter_context(tc.tile_pool(name="wdp", bufs=8))
        sqp = es.enter_context(tc.tile_pool(name="sq", bufs=4))
        smallp = es.enter_context(tc.tile_pool(name="small", bufs=4))
        probsp = es.enter_context(tc.tile_pool(name="probs", bufs=4))
        ptp = es.enter_context(tc.tile_pool(name="ptp", bufs=3))
        attnp = es.enter_context(tc.tile_pool(name="attn", bufs=2))
        mtilep = es.enter_context(tc.tile_pool(name="mtile", bufs=4))
        dchp = es.enter_context(tc.tile_pool(name="dch", bufs=2))
        arpool = es.enter_context(tc.tile_pool(name="arp", bufs=3))
        outp = es.enter_context(tc.tile_pool(name="outp", bufs=3))
        invp = es.enter_context(tc.tile_pool(name="invp", bufs=3))
        pp = es.enter_context(tc.tile_pool(name="psum", bufs=8, space="PSUM"))
        dram = es.enter_context(tc.tile_pool(name="dram", bufs=1, space="DRAM"))
        if True:
            # ---------------- collective warmup (first!) ----------------
            warm_src = smallp.tile([128, 16], BF16)
            nc.vector.memset(warm_src[:], 0.0)
            warm_in = dram.tile([128, 16], BF16)
            warm_out = dram.tile([128, 16], BF16, addr_space="Shared")
            nc.sync.dma_start(out=warm_in[:], in_=warm_src[:])
            nc.gpsimd.collective_compute(
                "AllReduce", ALU.add, replica_groups=RG,
                ins=[warm_in.opt()], outs=[warm_out.opt()])
            warm_sb = smallp.tile([1, 16], BF16)
            nc.sync.dma_start(out=warm_sb[:], in_=warm_out[0:1, :])

            # ---------------- persistent tiles ----------------
            X = {}    # f32 master residual stream
            XB = {}   # bf16 copy fed to matmuls
            for b in range(B):
                X[b] = persist.tile([128, NDC * S], F32, name=f"Xb{b}",
                                    tag=f"Xb{b}")
                XB[b] = persist.tile([128, NDC * S], BF16, name=f"XBb{b}",
                                     tag=f"XBb{b}")
            ident = persist.tile([128, 128], BF16)
            masks.make_identity(nc, ident[:])
            ones_col = persist.tile([128, 1], BF16)
            nc.vector.memset(ones_col[:], 1.0)
            ones_row = persist.tile([1, 128], F32)
            nc.vector.memset(ones_row[:], 1.0)
            eps_t = persist.tile([1, 1], F32)
            nc.vector.memset(eps_t[:], EPS)
            sin_sb = persist.tile([64, B * S], F32)
            cos_sb = persist.tile([64, B * S], F32)
            smask_sb = persist.tile([S, B * KA], BF16)
            cmask_sb = persist.tile([S, B * KT], BF16)
            fnw_sb = persist.tile([128, NDC], F32)

            for b in range(B):
                nc.sync.dma_start(
                    out=X[b][:].rearrange("p (c s) -> p c s", s=S),
                    in_=x_in[:, b * S:(b + 1) * S].rearrange(
                        "(c p) s -> p c s", p=128))
                for g in range(4):
                    nc.vector.tensor_copy(
                        XB[b][:, g * 4 * S:(g + 1) * 4 * S],
                        X[b][:, g * 4 * S:(g + 1) * 4 * S])
            nc.sync.dma_start(out=sin_sb[:], in_=sinT[:])
            nc.sync.dma_start(out=cos_sb[:], in_=cosT[:])
            nc.sync.dma_start(out=smask_sb[:], in_=smask[:])
            nc.sync.dma_start(out=cmask_sb[:], in_=cmask[:])
            nc.sync.dma_start(out=fnw_sb[:], in_=fnw[:])

            # AR bounce buffers, one pair per (reduction point, batch)
            ar_bufs = {}
            for k in range(3 * L):
                for b in range(B):
                    ar_bufs[(k, b)] = (
                        dram.tile([D, S], BF16, name=f"arin{k}_{b}", tag=f"arin{k}_{b}"),
                        dram.tile([D, S], BF16, name=f"arout{k}_{b}", tag=f"arout{k}_{b}",
                                  addr_space="Shared"),
                    )

            # ---------------- helpers ----------------
            def residual(slot, b, last=False):
                """Read back AR output for (slot, b); update XB (bf16, hot
                path for the next GEMMs) first, then the f32 master X."""
                arout = ar_bufs[(slot, b)][1]
                art = arpool.tile([128, NDC * S], BF16, tag="ar", bufs=1)
                nc.sync.dma_start(
                    out=art[:].rearrange("p (c s) -> p c s", s=S),
                    in_=arout[:].rearrange("(c p) s -> p c s", p=128))
                if last:
                    for g in range(4):
                        sl = slice(g * 4 * S, (g + 1) * 4 * S)
                        nc.vector.tensor_add(X[b][:, sl], X[b][:, sl], art[:, sl])
                    return
                for g in range(4):
                    sl = slice(g * 4 * S, (g + 1) * 4 * S)
                    nc.vector.tensor_add(XB[b][:, sl], X[b][:, sl], art[:, sl])
                for g in range(4):
                    sl = slice(g * 4 * S, (g + 1) * 4 * S)
                    nc.vector.tensor_add(X[b][:, sl], X[b][:, sl], art[:, sl])

            def stats_invb(b):
                """[128, S] broadcast tile of 1/rms(X_new[:, b]) from XB."""
                nw = pp.tile([1, 4 * S], F32, tag="psum", name=f"nw{nc.next_id()}")
                for g in range(4):
                    sq = sqp.tile([128, 4 * S], BF16, tag="sq", bufs=1)
                    nc.scalar.activation(sq[:], XB[b][:, g * 4 * S:(g + 1) * 4 * S],
                                         AF.Square)
                    nc.tensor.matmul(nw[:], ones_col[:], sq[:],
                                     start=(g == 0), stop=(g == 3))
                nwsb = smallp.tile([1, 4 * S], F32, tag="nwsb", bufs=1)
                nc.vector.tensor_copy(nwsb[:], nw[:])
                t0 = smallp.tile([1, S], F32, tag="nfold", bufs=3)
                t1 = smallp.tile([1, S], F32, tag="nfold", bufs=3)
                nc.vector.tensor_add(t0[:], nwsb[:, 0:S], nwsb[:, S:2 * S])
                nc.vector.tensor_add(t1[:], nwsb[:, 2 * S:3 * S], nwsb[:, 3 * S:4 * S])
                nstat = smallp.tile([1, S], F32, tag="nfold", bufs=3)
                nc.vector.tensor_add(nstat[:], t0[:], t1[:])
                sd = smallp.tile([1, S], F32, tag="sd")
                nc.scalar.activation(sd[:], nstat[:], AF.Sqrt,
                                     bias=eps_t[:], scale=1.0 / D)
                inv = smallp.tile([1, S], F32, tag="inv")
                nc.vector.reciprocal(inv[:], sd[:])
                invps = pp.tile([128, S], F32, tag="psum", name=f"inb{nc.next_id()}")
                nc.tensor.matmul(invps[:], ones_row[:], inv[:], start=True, stop=True)
                invb = invp.tile([128, S], F32, tag="invb", bufs=2)
                nc.vector.tensor_copy(invb[:], invps[:])
                return invb

            def scaled_sincos(b, invb):
                sn = smallp.tile([64, S], F32, tag="ssc", bufs=2)
                cs = smallp.tile([64, S], F32, tag="ssc", bufs=2)
                nc.vector.tensor_mul(sn[:], sin_sb[:, b * S:(b + 1) * S],
                                     invb[0:64, :])
                nc.vector.tensor_mul(cs[:], cos_sb[:, b * S:(b + 1) * S],
                                     invb[0:64, :])
                return sn, cs

            def rope_into(dst, dst_col, src_ap, sn, cs):
                t1 = smallp.tile([64, S], F32, tag="ropet", bufs=4)
                t2 = smallp.tile([64, S], F32, tag="ropet", bufs=4)
                x1 = src_ap[0:64, 0:S]
                x2 = src_ap[64:128, 0:S]
                nc.vector.tensor_mul(t1[:], cs[:], x1)
                nc.vector.tensor_mul(t2[:], sn[:], x2)
                nc.vector.tensor_sub(dst[0:64, dst_col:dst_col + S], t1[:], t2[:])
                t3 = smallp.tile([64, S], F32, tag="ropet", bufs=4)
                t4 = smallp.tile([64, S], F32, tag="ropet", bufs=4)
                nc.vector.tensor_mul(t3[:], cs[:], x2)
                nc.vector.tensor_mul(t4[:], sn[:], x1)
                nc.vector.tensor_add(dst[64:128, dst_col:dst_col + S], t3[:], t4[:])

            def ar_issue(slot, b, delta_sb):
                arin, arout = ar_bufs[(slot, b)]
                nc.sync.dma_start(
                    out=arin[:].rearrange("(c p) s -> p c s", p=128),
                    in_=delta_sb[:].rearrange("p (c s) -> p c s", s=S))
                nc.gpsimd.collective_compute(
                    "AllReduce", ALU.add, replica_groups=RG,
                    ins=[arin.opt()], outs=[arout.opt()])

            def second_proj(wts, act_sb, n_e):
                """delta[d, s] = sum_e W[e, d] act[e, s]; act_sb [128, n_e*S]
                cols h*S+s. wts: list of n_e weight tiles [128(e), D]."""
                delta_sb = dchp.tile([128, NDC * S], BF16, tag="dsb", bufs=2)
                for dc in range(NDC):
                    pd = pp.tile([128, S], F32, tag="psum", name=f"pd{nc.next_id()}")
                    for ec in range(n_e):
                        nc.tensor.matmul(
                            pd[:], wts[ec][:, dc * 128:(dc + 1) * 128],
                            act_sb[:, ec * S:(ec + 1) * S],
                            start=(ec == 0), stop=(ec == n_e - 1))
                    nc.scalar.activation(
                        delta_sb[:, dc * S:(dc + 1) * S], pd[:], AF.Copy)
                return delta_sb

            def load_layer_caches(l):
                st = {}
                for b in range(B):
                    kt = cachep.tile([128, KA], BF16, tag=f"kT{b}", bufs=1)
                    nc.sync.dma_start(out=kt[:], in_=kT[l, b])
                    st[("kT", b)] = kt
                    vt = cachep.tile([128, nkc * 128], BF16, tag=f"vTb{b}", bufs=1)
                    nc.sync.dma_start(
                        out=vt[:].rearrange("p (c f) -> p c f", f=128),
                        in_=vT[l, b].rearrange("(c p) f -> p c f", p=128))
                    st[("vTb", b)] = vt
                    for h in range(CH):
                        ck = cachep.tile([128, KT], BF16, tag=f"ckT{b}_{h}", bufs=1)
                        nc.sync.dma_start(out=ck[:], in_=ckT[l, b, h])
                        st[("ckT", b, h)] = ck
                        cv = cachep.tile([128, ntck * 128], BF16, tag=f"cvTb{b}_{h}", bufs=1)
                        nc.sync.dma_start(
                            out=cv[:].rearrange("p (c f) -> p c f", f=128),
                            in_=cvT[l, b, h].rearrange("(c p) f -> p c f", p=128))
                        st[("cvTb", b, h)] = cv
                return st

            def softmax_rows(ps_list, widths, p, mask_sb, mask_off):
                """exp+normalize a row-softmax split over groups.
                ps_list[g]: psum [S, widths[g]]; p: sbuf [S, sum(widths)] out."""
                dparts = []
                col = 0
                for g, ps in enumerate(ps_list):
                    wdt = widths[g]
                    nc.vector.tensor_add(
                        ps[:], ps[:],
                        mask_sb[:, mask_off + col:mask_off + col + wdt])
                    dp_ = smallp.tile([S, 1], F32, tag="denom", bufs=8)
                    nc.scalar.activation(p[:, col:col + wdt], ps[:], AF.Exp,
                                         accum_out=dp_[:])
                    dparts.append(dp_)
                    col += wdt
                denom = dparts[0]
                for dp_ in dparts[1:]:
                    dnew = smallp.tile([S, 1], F32, tag="denom", bufs=8)
                    nc.vector.tensor_add(dnew[:], denom[:], dp_[:])
                    denom = dnew
                invd = smallp.tile([S, 1], F32, tag="invd", bufs=4)
                nc.vector.reciprocal(invd[:], denom[:])
                nc.vector.tensor_scalar_mul(p[:], p[:], invd[:])

            def sa_block(l, b, st):
                invb = stats_invb(b)
                if b == 0:
                    qw_sb = wbig.tile([128, NDC * QH * HD], BF16, tag="wq", bufs=2)
                    nc.scalar.dma_start(
                        out=qw_sb[:].rearrange("p (c e) -> p c e", e=QH * HD),
                        in_=qw[l].rearrange("(c p) e -> p c e", p=128))
                    kv_sb = wbig.tile([128, NDC * 2 * HD], BF16, tag="wkv", bufs=1)
                    nc.scalar.dma_start(
                        out=kv_sb[:].rearrange("p (c e) -> p c e", e=2 * HD),
                        in_=kvw[l].rearrange("(c p) e -> p c e", p=128))
                    st["qw"], st["kv"] = qw_sb, kv_sb
                qw_sb, kv_sb = st["qw"], st["kv"]

                pq0 = pp.tile([128, S], F32, tag="psum", name=f"pq0{nc.next_id()}")
                pq1 = pp.tile([128, S], F32, tag="psum", name=f"pq1{nc.next_id()}")
                pk = pp.tile([128, S], F32, tag="psum", name=f"pk{nc.next_id()}")
                pv = pp.tile([128, S], F32, tag="psum", name=f"pv{nc.next_id()}")
                for i in range(NDC):
                    stt, spp = (i == 0), (i == NDC - 1)
                    qo, ko = i * QH * HD, i * 2 * HD
                    hsl = XB[b][:, i * S:(i + 1) * S]
                    nc.tensor.matmul(pq0[:], qw_sb[:, qo:qo + 128], hsl, start=stt, stop=spp)
                    nc.tensor.matmul(pq1[:], qw_sb[:, qo + 128:qo + 256], hsl, start=stt, stop=spp)
                    nc.tensor.matmul(pk[:], kv_sb[:, ko:ko + 128], hsl, start=stt, stop=spp)
                    nc.tensor.matmul(pv[:], kv_sb[:, ko + 128:ko + 256], hsl, start=stt, stop=spp)

                sn, cs = scaled_sincos(b, invb)
                q_roped = probsp.tile([128, QH * S], BF16, tag="qrope", bufs=2)
                rope_into(q_roped, 0, pq0[:], sn, cs)
                rope_into(q_roped, S, pq1[:], sn, cs)
                rope_into(st[("kT", b)], w, pk[:], sn, cs)

                vsb = probsp.tile([128, S], BF16, tag="vsb", bufs=2)
                nc.vector.tensor_mul(vsb[:], pv[:], invb[:])
                pvt = pp.tile([128, 128], BF16, tag="psum", name=f"pvt{nc.next_id()}")
                nc.tensor.transpose(pvt[:], vsb[:], ident[:])
                r, c0 = w % 128, w // 128
                vtb = st[("vTb", b)]
                if r == 0:
                    nc.vector.tensor_copy(vtb[:, c0 * 128:(c0 + 1) * 128], pvt[:])
                else:
                    nc.vector.tensor_copy(vtb[r:128, c0 * 128:(c0 + 1) * 128],
                                          pvt[0:128 - r, :])
                    nc.vector.tensor_copy(vtb[0:r, (c0 + 1) * 128:(c0 + 2) * 128],
                                          pvt[128 - r:128, :])

                p_tiles = {}
                for h in range(QH):
                    p = probsp.tile([S, KA], BF16, tag="p", bufs=2)
                    ps_list, widths = [], []
                    for g0 in range(0, nkc, 4):
                        wdt = min(4, nkc - g0) * 128
                        ps = pp.tile([S, wdt], F32, tag="psum", name=f"ps{nc.next_id()}")
                        nc.tensor.matmul(
                            ps[:], q_roped[:, h * S:(h + 1) * S],
                            st[("kT", b)][:, g0 * 128:g0 * 128 + wdt],
                            start=True, stop=True)
                        ps_list.append(ps)
                        widths.append(wdt)
                    softmax_rows(ps_list, widths, p, smask_sb, b * KA)
                    p_tiles[h] = p

                pattn = pp.tile([128, QH * S], F32, tag="psum", name=f"pat{nc.next_id()}")
                for j in range(nkc):
                    pT = ptp.tile([128, QH * S], BF16, tag="pT", bufs=2)
                    for h in range(QH):
                        ptps = pp.tile([S, 128], BF16, tag="psum", name=f"ptp{nc.next_id()}")
                        nc.tensor.transpose(
                            ptps[:], p_tiles[h][:, j * 128:(j + 1) * 128], ident[:])
                        nc.vector.tensor_copy(pT[:, h * S:(h + 1) * S], ptps[:])
                    nc.tensor.matmul(
                        pattn[:], st[("vTb", b)][:, j * 128:(j + 1) * 128], pT[:],
                        start=(j == 0), stop=(j == nkc - 1))
                attn_sb = attnp.tile([128, QH * S], BF16, tag="attn")
                nc.scalar.activation(attn_sb[:], pattn[:], AF.Copy)

                if b == 0:
                    wts = []
                    for ec in range(QH):
                        wt = wrow.tile([128, D], BF16, tag="wrow", bufs=2)
                        nc.scalar.dma_start(out=wt[:], in_=ow[l, ec * 128:(ec + 1) * 128, :])
                        wts.append(wt)
                    st["ow"] = wts
                return second_proj(st["ow"], attn_sb, QH)

            def ca_block(l, b, st):
                invb = stats_invb(b)
                if b == 0:
                    cq_sb = wbig.tile([128, NDC * CH * HDC], BF16, tag="wq", bufs=2)
                    nc.scalar.dma_start(
                        out=cq_sb[:].rearrange("p (c e) -> p c e", e=CH * HDC),
                        in_=cqw[l].rearrange("(c p) e -> p c e", p=128))
                    st["cq"] = cq_sb
                cq_sb = st["cq"]
                pcq0 = pp.tile([128, S], F32, tag="psum", name=f"pcq0{nc.next_id()}")
                pcq1 = pp.tile([128, S], F32, tag="psum", name=f"pcq1{nc.next_id()}")
                for i in range(NDC):
                    stt, spp = (i == 0), (i == NDC - 1)
                    qo = i * CH * HDC
                    hsl = XB[b][:, i * S:(i + 1) * S]
                    nc.tensor.matmul(pcq0[:], cq_sb[:, qo:qo + 128], hsl, start=stt, stop=spp)
                    nc.tensor.matmul(pcq1[:], cq_sb[:, qo + 128:qo + 256], hsl, start=stt, stop=spp)
                sn, cs = scaled_sincos(b, invb)
                cq_roped = probsp.tile([128, CH * S], BF16, tag="qrope", bufs=2)
                rope_into(cq_roped, 0, pcq0[:], sn, cs)
                rope_into(cq_roped, S, pcq1[:], sn, cs)

                cattn_sb = attnp.tile([128, CH * S], BF16, tag="attn")
                for h in range(CH):
                    ps = pp.tile([S, KT], F32, tag="psum", name=f"cps{nc.next_id()}")
                    nc.tensor.matmul(
                        ps[:], cq_roped[:, h * S:(h + 1) * S],
                        st[("ckT", b, h)][:], start=True, stop=True)
                    p = probsp.tile([S, KT], BF16, tag="cp", bufs=2)
                    softmax_rows([ps], [KT], p, cmask_sb, b * KT)
                    pcat = pp.tile([128, S], F32, tag="psum", name=f"pca{nc.next_id()}")
                    for j in range(ntck):
                        pT = ptp.tile([S, 128], BF16, tag="cpT", bufs=2)
                        ptps = pp.tile([S, 128], BF16, tag="psum", name=f"ptc{nc.next_id()}")
                        nc.tensor.transpose(
                            ptps[:], p[:, j * 128:(j + 1) * 128], ident[:])
                        nc.vector.tensor_copy(pT[:], ptps[:])
                        nc.tensor.matmul(
                            pcat[:], st[("cvTb", b, h)][:, j * 128:(j + 1) * 128], pT[:],
                            start=(j == 0), stop=(j == ntck - 1))
                    nc.scalar.activation(cattn_sb[:, h * S:(h + 1) * S],
                                         pcat[:], AF.Copy)

                if b == 0:
                    wts = []
                    for ec in range(CH):
                        wt = wrow.tile([128, D], BF16, tag="wrow", bufs=2)
                        nc.scalar.dma_start(out=wt[:], in_=cow[l, ec * 128:(ec + 1) * 128, :])
                        wts.append(wt)
                    st["cow"] = wts
                return second_proj(st["cow"], cattn_sb, CH)

            def mlp_block_b(l, b, st):
                """MLP for one batch; weights loaded at b==0 stay resident."""
                invb = stats_invb(b)
                WW = 2
                if b == 0:
                    gts, uts = [], []
                    for wv in range(NFC // WW):
                        gt = wmlp.tile([128, NDC * WW * 128], BF16,
                                       tag=f"wg{wv}", bufs=1)
                        nc.scalar.dma_start(
                            out=gt[:].rearrange("p (c f) -> p c f", f=WW * 128),
                            in_=wgw[l, :, wv * WW * 128:(wv + 1) * WW * 128].rearrange(
                                "(c p) f -> p c f", p=128))
                        ut = wmlp.tile([128, NDC * WW * 128], BF16,
                                       tag=f"wu{wv}", bufs=1)
                        nc.scalar.dma_start(
                            out=ut[:].rearrange("p (c f) -> p c f", f=WW * 128),
                            in_=wuw[l, :, wv * WW * 128:(wv + 1) * WW * 128].rearrange(
                                "(c p) f -> p c f", p=128))
                        gts.append(gt)
                        uts.append(ut)
                    st["wg"], st["wu"] = gts, uts
                    wd_sb = []
                    for dhalf in range(2):
                        for fg in range(2):
                            t = wdp.tile([128, 4 * (D // 2)], BF16,
                                         tag=f"wd{dhalf}_{fg}", bufs=1)
                            nc.scalar.dma_start(
                                out=t[:].rearrange("p (c f) -> p c f", f=D // 2),
                                in_=wdw[l, fg * 512:(fg + 1) * 512,
                                        dhalf * (D // 2):(dhalf + 1) * (D // 2)].rearrange(
                                    "(c p) f -> p c f", p=128))
                            wd_sb.append(t)
                    st["wd"] = wd_sb
                gts, uts, wd_sb = st["wg"], st["wu"], st["wd"]

                m_tiles = []
                for wv in range(NFC // WW):
                    gt, ut = gts[wv], uts[wv]
                    for k in range(WW):
                        pg = pp.tile([128, S], F32, tag="psum", name=f"pg{nc.next_id()}")
                        pu = pp.tile([128, S], F32, tag="psum", name=f"pu{nc.next_id()}")
                        for i in range(NDC):
                            stt, spp = (i == 0), (i == NDC - 1)
                            co = i * WW * 128 + k * 128
                            hsl = XB[b][:, i * S:(i + 1) * S]
                            nc.tensor.matmul(pg[:], gt[:, co:co + 128],
                                             hsl, start=stt, stop=spp)
                            nc.tensor.matmul(pu[:], ut[:, co:co + 128],
                                             hsl, start=stt, stop=spp)
                        sgs = mtilep.tile([128, S], BF16, tag="sgs", bufs=2)
                        nc.vector.tensor_mul(sgs[:], pg[:], invb[:])
                        sg = mtilep.tile([128, S], BF16, tag="sg")
                        nc.scalar.activation(sg[:], sgs[:], AF.Silu)
                        ub = mtilep.tile([128, S], BF16, tag="ub")
                        nc.vector.tensor_mul(ub[:], pu[:], invb[:])
                        m = mtilep.tile([128, S], BF16, tag="m", bufs=12)
                        nc.vector.tensor_mul(m[:], sg[:], ub[:])
                        m_tiles.append(m)

                delta = dchp.tile([128, NDC * S], BF16, tag="dsb", bufs=2,
                                  name=f"dmlp{l}_{b}")
                for dhalf in range(2):
                    for dc8 in range(NDC // 2):
                        dc = dhalf * (NDC // 2) + dc8
                        pd = pp.tile([128, S], F32, tag="psum", name=f"pdm{nc.next_id()}")
                        for fc in range(NFC):
                            t = wd_sb[dhalf * 2 + fc // 4]
                            co = (fc % 4) * (D // 2) + dc8 * 128
                            nc.tensor.matmul(
                                pd[:], t[:, co:co + 128],
                                m_tiles[fc][:], start=(fc == 0), stop=(fc == NFC - 1))
                        nc.scalar.activation(
                            delta[:, dc * S:(dc + 1) * S], pd[:], AF.Copy)
                return delta

            # ================= layers (batch-staggered pipeline) =========
            for l in range(L):
                st = load_layer_caches(l)
                # SA
                for b in range(B):
                    if l > 0:
                        residual(3 * (l - 1) + 2, b)
                    d = sa_block(l, b, st)
                    ar_issue(3 * l + 0, b, d)
                # CA
                for b in range(B):
                    residual(3 * l + 0, b)
                    d = ca_block(l, b, st)
                    ar_issue(3 * l + 1, b, d)
                # MLP (batch-sequential: AR of b0 hides under b1 compute)
                for b in range(B):
                    residual(3 * l + 1, b)
                    d = mlp_block_b(l, b, st)
                    ar_issue(3 * l + 2, b, d)

            # ================ final norm + output ================
            nc.vector.tensor_scalar_mul(warm_sb[:], warm_sb[:], 0.0)
            for b in range(B):
                residual(3 * (L - 1) + 2, b, last=True)
                nw = pp.tile([1, 4 * S], F32, tag="psum", name=f"nsf{b}")
                for g in range(4):
                    sq = sqp.tile([128, 4 * S], BF16, tag="sq", bufs=1)
                    nc.scalar.activation(sq[:], X[b][:, g * 4 * S:(g + 1) * 4 * S],
                                         AF.Square)
                    nc.tensor.matmul(nw[:], ones_col[:], sq[:],
                                     start=(g == 0), stop=(g == 3))
                nwsb = smallp.tile([1, 4 * S], F32, tag="nwsb", bufs=1)
                nc.vector.tensor_copy(nwsb[:], nw[:])
                t0 = smallp.tile([1, S], F32, tag="nfold", bufs=3)
                t1 = smallp.tile([1, S], F32, tag="nfold", bufs=3)
                nc.vector.tensor_add(t0[:], nwsb[:, 0:S], nwsb[:, S:2 * S])
                nc.vector.tensor_add(t1[:], nwsb[:, 2 * S:3 * S], nwsb[:, 3 * S:4 * S])
                nstat = smallp.tile([1, S], F32, tag="nfold", bufs=3)
                nc.vector.tensor_add(nstat[:], t0[:], t1[:])
                sd = smallp.tile([1, S], F32, tag="sd")
                nc.scalar.activation(sd[:], nstat[:], AF.Sqrt,
                                     bias=eps_t[:], scale=1.0 / D)
                inv = smallp.tile([1, S], F32, tag="inv")
                nc.vector.reciprocal(inv[:], sd[:])
                invps = pp.tile([128, S], F32, tag="psum", name=f"inf{b}")
                nc.tensor.matmul(invps[:], ones_row[:], inv[:], start=True, stop=True)
                invb = invp.tile([128, S], F32, tag="invb", bufs=2)
                nc.vector.tensor_copy(invb[:], invps[:])
                for g in range(4):
                    sl = slice(g * 4 * S, (g + 1) * 4 * S)
                    nc.vector.tensor_mul(X[b][:, sl], X[b][:, sl],
                                         invb[:].rearrange("p (o s) -> p o s", o=1)
                                         .to_broadcast((128, 4, S)))
                    nc.vector.tensor_mul(
                        X[b][:, sl].rearrange("p (c s) -> p c s", s=S),
                        X[b][:, sl].rearrange("p (c s) -> p c s", s=S),
                        fnw_sb[:, g * 4:(g + 1) * 4].rearrange("p (c o) -> p c o", o=1)
                        .to_broadcast((128, 4, S)))
                if b == 0:
                    nc.vector.tensor_add(X[b][0:1, 0:16], X[b][0:1, 0:16],
                                         warm_sb[:])
                nc.sync.dma_start(
                    out=out_ext[:, b * S:(b + 1) * S].rearrange(
                        "(c p) s -> p c s", p=128),
                    in_=X[b][:].rearrange("p (c s) -> p c s", s=S))

    nc.finalize()
    return nc


# ---------------------------------------------------------------- host prep
def _prep_in_maps(inputs):
    f32 = np.float32
    x = inputs["x"].astype(f32)                      # (B, D, 1, S)
    positions = inputs["positions"]
    w = int(np.asarray(inputs["kv_write_index"]).reshape(-1)[0])
    self_attn_mask = inputs["self_attn_mask"].astype(f32)  # (B,1,S,A)
    enc_len = np.asarray(inputs["encoder_lengths"]).reshape(B)

    sa_n = inputs["sa_norm_w"].astype(f32)[:, :, None]     # (L, D, 1)
    ca_n = inputs["ca_norm_w"].astype(f32)[:, :, None]
    mlp_n = inputs["mlp_norm_w"].astype(f32)[:, :, None]
    scale = 1.0 / np.sqrt(HD).astype(f32)
    cscale = 1.0 / np.sqrt(HDC).astype(f32)

    qw = (inputs["q_w"] * sa_n * scale).astype(BF)         # (L, D, HQ*HD)
    kw = (inputs["k_w"] * sa_n).astype(BF)
    vw = (inputs["v_w"] * sa_n).astype(BF)
    ow = inputs["o_w"].astype(BF)                          # (L, HQ*HD, D)
    cqw = (inputs["cq_w"] * ca_n * cscale).astype(BF)
    cow = inputs["co_w"].astype(BF)
    wgw = (inputs["wg_w"] * mlp_n).astype(BF)
    wuw = (inputs["wu_w"] * mlp_n).astype(BF)
    wdw = inputs["wd_w"].astype(BF)

    # --- masked-chunk classification -------------------------------------
    # self-attn: chunk c of the A axis is skippable if fully masked for all
    # queries in both batches; visible chunks form a prefix for causal masks.
    m = self_attn_mask[:, 0]                               # (B, S, A)
    ch = m.reshape(B, S, A // 128, 128)
    full_masked = (ch < -1e3).all(axis=(0, 1, 3))          # (A//128,)
    keep = np.nonzero(~full_masked)[0]
    if len(keep) == 0:
        nkc = A // 128
        m = m + 3e4
    else:
        nkc = int(keep[-1]) + 1
    # ensure the KV write region is covered
    nkc = max(nkc, (w + S + 127) // 128)
    nkc = min(nkc, A // 128)
    KA = nkc * 128

    # cross-attn: valid keys are t < enc_len (prefix); fully-masked batch
    # falls back to all chunks with a shifted (all-zero) mask == uniform.
    if (enc_len <= 0).any():
        ntck = T // 128
    else:
        ntck = int((enc_len.max() + 127) // 128)
    KT = ntck * 128

    k_cache = inputs["k_cache"].reshape(L, B, HKV, A, HD)
    v_cache = inputs["v_cache"].reshape(L, B, HKV, HD, A)
    ck = inputs["ck_cache"].reshape(L, B, HC, T, HDC)
    cv = inputs["cv_cache"].reshape(L, B, HC, HDC, T)
    kTf = np.ascontiguousarray(
        k_cache.transpose(0, 1, 2, 4, 3)[:, :, :, :, :KA]).astype(BF)
    vTf = np.ascontiguousarray(
        v_cache.transpose(0, 1, 2, 4, 3)[:, :, :, :KA, :]).astype(BF)
    ckTf = np.ascontiguousarray(
        ck.transpose(0, 1, 2, 4, 3)[:, :, :, :, :KT]).astype(BF)
    cvTf = np.ascontiguousarray(
        cv.transpose(0, 1, 2, 4, 3)[:, :, :KT, :]).astype(BF)

    inv_freq = 1.0 / (10000.0 ** (np.arange(0, HD, 2, dtype=f32) / HD))
    ang = positions.astype(f32)[:, None, :] * inv_freq[None, :, None]   # (B,64,S)
    sinT = np.ascontiguousarray(np.sin(ang).transpose(1, 0, 2).reshape(64, B * S)).astype(f32)
    cosT = np.ascontiguousarray(np.cos(ang).transpose(1, 0, 2).reshape(64, B * S)).astype(f32)

    smask = np.ascontiguousarray(
        m[:, :, :KA].transpose(1, 0, 2).reshape(S, B * KA)).astype(BF)
    t_idx = np.arange(KT)
    cm = np.where(t_idx[None, :] < enc_len[:, None], 0.0, NEG).astype(f32)
    for b in range(B):
        if enc_len[b] <= 0:
            cm[b] += 3e4
    cmask = np.ascontiguousarray(
        np.broadcast_to(cm.reshape(1, B * KT), (S, B * KT))).astype(BF)

    x_in = np.ascontiguousarray(
        x[:, :, 0, :].transpose(1, 0, 2).reshape(D, B * S)).astype(f32)
    fnw = np.ascontiguousarray(
        inputs["final_norm_w"].astype(f32).reshape(NDC, 128).T)

    in_maps = []
    for c in range(NCORES):
        qh = slice(2 * c * HD, (2 * c + 2) * HD)
        kvh = c // 2
        ffs = slice(c * FFS, (c + 1) * FFS)
        in_maps.append({
            "x_in": x_in,
            "qw": np.ascontiguousarray(qw[:, :, qh]),
            "kvw": np.ascontiguousarray(np.concatenate(
                [kw[:, :, kvh * HD:(kvh + 1) * HD],
                 vw[:, :, kvh * HD:(kvh + 1) * HD]], axis=2)),
            "ow": np.ascontiguousarray(ow[:, qh, :]),
            "cqw": np.ascontiguousarray(cqw[:, :, qh]),
            "cow": np.ascontiguousarray(cow[:, qh, :]),
            "wgw": np.ascontiguousarray(wgw[:, :, ffs]),
            "wuw": np.ascontiguousarray(wuw[:, :, ffs]),
            "wdw": np.ascontiguousarray(wdw[:, ffs, :]),
            "kT": np.ascontiguousarray(kTf[:, :, kvh]),
            "vT": np.ascontiguousarray(vTf[:, :, kvh]),
            "ckT": np.ascontiguousarray(ckTf[:, :, 2 * c:2 * c + 2]),
            "cvT": np.ascontiguousarray(cvTf[:, :, 2 * c:2 * c + 2]),
            "sinT": sinT, "cosT": cosT,
            "smask": smask, "cmask": cmask,
            "fnw": fnw,
        })
    return in_maps, w, nkc, ntck


def kernel(**inputs):
    global _exec_time_ns
    in_maps, w, nkc, ntck = _prep_in_maps(inputs)
    nc = build_graph(w, nkc, ntck)
    trace = bool(int(os.environ.get("BASS_KERNEL_TRACE", "0")))
    res = run_bass_kernel_spmd(nc, in_maps, list(range(NCORES)), trace=trace)
    _exec_time_ns = res.exec_time_ns
    out = np.asarray(res.results[0]["out"])          # [D, B*S] f32
    out = out.reshape(D, B, S).transpose(1, 0, 2)[:, :, None, :]
    return np.ascontiguousarray(out.astype(np.float32))


# revision 16
# speedup vs baseline: 1.1134x; 1.0260x over previous
"""Trainium2 Bass kernel for nn_ANEDecoder (Dia-style ANE decoder, 2 layers).

Sharding: tensor-parallel across 8 cores — 2 query heads + their (shared)
KV head per core for self-attn, 2 cross heads per core, FF/8 = 1024 MLP
hidden per core, all-reduce after o_proj / co_proj / wd.

v3 over v2:
- Post-scale RMSNorm: blocks matmul the *un-normalized* residual (XB, bf16)
  immediately after the all-reduce lands; the 1/rms factor is applied to the
  (much smaller) projection outputs, so the stats chain overlaps the GEMMs
  instead of serializing in front of them.
- Masked-chunk skipping: self-attention only touches the ceil((w+S)/128)
  cache chunks the causal mask can reach (3 of 12 for w=256); cross-attn
  only ceil(enc_len/128) chunks. Fully-masked chunks contribute exp(-3e4)=0
  exactly, so skipping is lossless.
- AR readback on the sync queue (gpsimd queue carries only collectives),
  residual add writes the bf16 GEMM input directly (f32 master copy updated
  off-path on gpsimd).
- Warmup collective issued as the first instruction so the one-time CC
  bring-up overlaps the prologue DMA.
"""
import os
import sys
import functools
from contextlib import ExitStack

sys.path.insert(0, "/opt/trn_rl_repo")

import numpy as np
import ml_dtypes

import concourse.bass as bass
import concourse.bacc as bacc
import concourse.mybir as mybir
import concourse.tile as tile
import concourse.masks as masks
from concourse.bass_utils import run_bass_kernel_spmd

BF = ml_dtypes.bfloat16
F32 = mybir.dt.float32
BF16 = mybir.dt.bfloat16
AF = mybir.ActivationFunctionType
ALU = mybir.AluOpType

# dims
B, D, S = 2, 2048, 128
A, T = 1536, 512
HQ, HKV, HD = 16, 4, 128
HC, HDC = 16, 128
FF, L = 8192, 2
EPS = 1e-5
NEG = -3e4

NCORES = 8
NDC = D // 128          # 16 d-chunks
QH = HQ // NCORES       # 2 query heads per core
CH = HC // NCORES       # 2 cross heads per core
FFS = FF // NCORES      # 1024 ff per core
NFC = FFS // 128        # 8 ff chunks

_exec_time_ns = None


def last_exec_time_ns():
    return _exec_time_ns


# ---------------------------------------------------------------- builder
@functools.lru_cache(maxsize=4)
def build_graph(w: int, nkc: int, ntck: int):
    KA = nkc * 128          # kept self-attn cache columns
    KT = ntck * 128         # kept cross-attn cache columns
    nc = bacc.Bacc()

    def par(name, shape, dt):
        return nc.declare_dram_parameter(name, list(shape), dt, isOutput=False)

    x_in = par("x_in", (D, B * S), F32)
    qw = par("qw", (L, D, QH * HD), BF16)
    kvw = par("kvw", (L, D, 2 * HD), BF16)
    ow = par("ow", (L, QH * HD, D), BF16)
    cqw = par("cqw", (L, D, CH * HDC), BF16)
    cow = par("cow", (L, CH * HDC, D), BF16)
    wgw = par("wgw", (L, D, FFS), BF16)
    wuw = par("wuw", (L, D, FFS), BF16)
    wdw = par("wdw", (L, FFS, D), BF16)
    kT = par("kT", (L, B, HD, KA), BF16)
    vT = par("vT", (L, B, KA, HD), BF16)
    ckT = par("ckT", (L, B, CH, HDC, KT), BF16)
    cvT = par("cvT", (L, B, CH, KT, HDC), BF16)
    sinT = par("sinT", (HD // 2, B * S), F32)
    cosT = par("cosT", (HD // 2, B * S), F32)
    smask = par("smask", (S, B * KA), BF16)
    cmask = par("cmask", (S, B * KT), BF16)
    fnw = par("fnw", (128, NDC), F32)
    out_ext = nc.declare_dram_parameter("out", [D, B * S], F32, isOutput=True)

    RG = [list(range(NCORES))]

    with tile.TileContext(nc) as tc, ExitStack() as es:
        persist = es.enter_context(tc.tile_pool(name="persist", bufs=1))
        cachep = es.enter_context(tc.tile_pool(name="cache", bufs=2))
        wbig = es.enter_context(tc.tile_pool(name="wbig", bufs=2))
        wrow = es.enter_context(tc.tile_pool(name="wrow", bufs=2))
        wmlp = es.enter_context(tc.tile_pool(name="wmlp", bufs=6))
        wdp = es.enter_context(tc.tile_pool(name="wdp", bufs=8))
        sqp = es.enter_context(tc.tile_pool(name="sq", bufs=4))
        smallp = es.enter_context(tc.tile_pool(name="small", bufs=4))
        probsp = es.enter_context(tc.tile_pool(name="probs", bufs=4))
        ptp = es.enter_context(tc.tile_pool(name="ptp", bufs=3))
        attnp = es.enter_context(tc.tile_pool(name="attn", bufs=2))
        mtilep = es.enter_context(tc.tile_pool(name="mtile", bufs=4))
        dchp = es.enter_context(tc.tile_pool(name="dch", bufs=2))
        arpool = es.enter_context(tc.tile_pool(name="arp", bufs=3))
        outp = es.enter_context(tc.tile_pool(name="outp", bufs=3))
        invp = es.enter_context(tc.tile_pool(name="invp", bufs=3))
        pp = es.enter_context(tc.tile_pool(name="psum", bufs=8, space="PSUM"))
        dram = es.enter_context(tc.tile_pool(name="dram", bufs=1, space="DRAM"))
        if True:
            # ---------------- collective warmup (first!) ----------------
            warm_src = smallp.tile([128, 16], BF16)
            nc.vector.memset(warm_src[:], 0.0)
            warm_in = dram.tile([128, 16], BF16)
            warm_out = dram.tile([128, 16], BF16, addr_space="Shared")
            nc.sync.dma_start(out=warm_in[:], in_=warm_src[:])
            nc.gpsimd.collective_compute(
                "AllReduce", ALU.add, replica_groups=RG,
                ins=[warm_in.opt()], outs=[warm_out.opt()])
            warm_sb = smallp.tile([1, 16], BF16)
            nc.sync.dma_start(out=warm_sb[:], in_=warm_out[0:1, :])

            # ---------------- persistent tiles ----------------
            X = {}    # f32 master residual stream
            XB = {}   # bf16 copy fed to matmuls
            for b in range(B):
                X[b] = persist.tile([128, NDC * S], F32, name=f"Xb{b}",
                                    tag=f"Xb{b}")
                XB[b] = persist.tile([128, NDC * S], BF16, name=f"XBb{b}",
                                     tag=f"XBb{b}")
            ident = persist.tile([128, 128], BF16)
            masks.make_identity(nc, ident[:])
            ones_col = persist.tile([128, 1], BF16)
            nc.vector.memset(ones_col[:], 1.0)
            ones_row = persist.tile([1, 128], F32)
            nc.vector.memset(ones_row[:], 1.0)
            eps_t = persist.tile([1, 1], F32)
            nc.vector.memset(eps_t[:], EPS)
            sin_sb = persist.tile([64, B * S], F32)
            cos_sb = persist.tile([64, B * S], F32)
            smask_sb = persist.tile([S, B * KA], BF16)
            cmask_sb = persist.tile([S, B * KT], BF16)
            fnw_sb = persist.tile([128, NDC], F32)

            for b in range(B):
                nc.sync.dma_start(
                    out=X[b][:].rearrange("p (c s) -> p c s", s=S),
                    in_=x_in[:, b * S:(b + 1) * S].rearrange(
                        "(c p) s -> p c s", p=128))
                for g in range(4):
                    nc.vector.tensor_copy(
                        XB[b][:, g * 4 * S:(g + 1) * 4 * S],
                        X[b][:, g * 4 * S:(g + 1) * 4 * S])
            nc.sync.dma_start(out=sin_sb[:], in_=sinT[:])
            nc.sync.dma_start(out=cos_sb[:], in_=cosT[:])
            nc.sync.dma_start(out=smask_sb[:], in_=smask[:])
            nc.sync.dma_start(out=cmask_sb[:], in_=cmask[:])
            nc.sync.dma_start(out=fnw_sb[:], in_=fnw[:])

            # AR bounce buffers, one pair per (reduction point, batch)
            ar_bufs = {}
            for k in range(3 * L):
                for b in range(B):
                    ar_bufs[(k, b)] = (
                        dram.tile([D, S], BF16, name=f"arin{k}_{b}", tag=f"arin{k}_{b}"),
                        dram.tile([D, S], BF16, name=f"arout{k}_{b}", tag=f"arout{k}_{b}",
                                  addr_space="Shared"),
                    )

            # ---------------- helpers ----------------
            def residual(slot, b, last=False):
                """Read back AR output for (slot, b); update XB (bf16, hot
                path for the next GEMMs) first, then the f32 master X."""
                arout = ar_bufs[(slot, b)][1]
                arts = []
                for h in range(2):
                    art = arpool.tile([128, 8 * S], BF16, tag="ar", bufs=2)
                    nc.sync.dma_start(
                        out=art[:].rearrange("p (c s) -> p c s", s=S),
                        in_=arout[h * 1024:(h + 1) * 1024, :].rearrange(
                            "(c p) s -> p c s", p=128))
                    arts.append(art)
                    for g in range(2 * h, 2 * h + 2):
                        sl = slice(g * 4 * S, (g + 1) * 4 * S)
                        asl = slice((g - 2 * h) * 4 * S, (g - 2 * h + 1) * 4 * S)
                        nc.vector.tensor_add(
                            X[b][:, sl] if last else XB[b][:, sl],
                            X[b][:, sl], art[:, asl])
                if last:
                    return
                for h in range(2):
                    for g in range(2 * h, 2 * h + 2):
                        sl = slice(g * 4 * S, (g + 1) * 4 * S)
                        asl = slice((g - 2 * h) * 4 * S, (g - 2 * h + 1) * 4 * S)
                        nc.vector.tensor_add(X[b][:, sl], X[b][:, sl],
                                             arts[h][:, asl])

            def stats_invb(b):
                """[128, S] broadcast tile of 1/rms(X_new[:, b]) from XB."""
                nw = pp.tile([1, 4 * S], F32, tag="psum", name=f"nw{nc.next_id()}")
                for g in range(4):
                    sq = sqp.tile([128, 4 * S], BF16, tag="sq", bufs=1)
                    nc.scalar.activation(sq[:], XB[b][:, g * 4 * S:(g + 1) * 4 * S],
                                         AF.Square)
                    nc.tensor.matmul(nw[:], ones_col[:], sq[:],
                                     start=(g == 0), stop=(g == 3))
                nwsb = smallp.tile([1, 4 * S], F32, tag="nwsb", bufs=1)
                nc.vector.tensor_copy(nwsb[:], nw[:])
                t0 = smallp.tile([1, S], F32, tag="nfold", bufs=3)
                t1 = smallp.tile([1, S], F32, tag="nfold", bufs=3)
                nc.vector.tensor_add(t0[:], nwsb[:, 0:S], nwsb[:, S:2 * S])
                nc.vector.tensor_add(t1[:], nwsb[:, 2 * S:3 * S], nwsb[:, 3 * S:4 * S])
                nstat = smallp.tile([1, S], F32, tag="nfold", bufs=3)
                nc.vector.tensor_add(nstat[:], t0[:], t1[:])
                sd = smallp.tile([1, S], F32, tag="sd")
                nc.scalar.activation(sd[:], nstat[:], AF.Sqrt,
                                     bias=eps_t[:], scale=1.0 / D)
                inv = smallp.tile([1, S], F32, tag="inv")
                nc.vector.reciprocal(inv[:], sd[:])
                invps = pp.tile([128, S], F32, tag="psum", name=f"inb{nc.next_id()}")
                nc.tensor.matmul(invps[:], ones_row[:], inv[:], start=True, stop=True)
                invb = invp.tile([128, S], F32, tag="invb", bufs=2)
                nc.vector.tensor_copy(invb[:], invps[:])
                return invb

            def scaled_sincos(b, invb):
                sn = smallp.tile([64, S], F32, tag="ssc", bufs=2)
                cs = smallp.tile([64, S], F32, tag="ssc", bufs=2)
                nc.vector.tensor_mul(sn[:], sin_sb[:, b * S:(b + 1) * S],
                                     invb[0:64, :])
                nc.vector.tensor_mul(cs[:], cos_sb[:, b * S:(b + 1) * S],
                                     invb[0:64, :])
                return sn, cs

            def rope_into(dst, dst_col, src_ap, sn, cs):
                t1 = smallp.tile([64, S], F32, tag="ropet", bufs=4)
                t2 = smallp.tile([64, S], F32, tag="ropet", bufs=4)
                x1 = src_ap[0:64, 0:S]
                x2 = src_ap[64:128, 0:S]
                nc.vector.tensor_mul(t1[:], cs[:], x1)
                nc.vector.tensor_mul(t2[:], sn[:], x2)
                nc.vector.tensor_sub(dst[0:64, dst_col:dst_col + S], t1[:], t2[:])
                t3 = smallp.tile([64, S], F32, tag="ropet", bufs=4)
                t4 = smallp.tile([64, S], F32, tag="ropet", bufs=4)
                nc.vector.tensor_mul(t3[:], cs[:], x2)
                nc.vector.tensor_mul(t4[:], sn[:], x1)
                nc.vector.tensor_add(dst[64:128, dst_col:dst_col + S], t3[:], t4[:])

            def ar_issue(slot, b, delta_sb):
                arin, arout = ar_bufs[(slot, b)]
                nc.sync.dma_start(
                    out=arin[:].rearrange("(c p) s -> p c s", p=128),
                    in_=delta_sb[:].rearrange("p (c s) -> p c s", s=S))
                nc.gpsimd.collective_compute(
                    "AllReduce", ALU.add, replica_groups=RG,
                    ins=[arin.opt()], outs=[arout.opt()])

            def second_proj(wts, act_sb, n_e):
                """delta[d, s] = sum_e W[e, d] act[e, s]; act_sb [128, n_e*S]
                cols h*S+s. wts: list of n_e weight tiles [128(e), D]."""
                delta_sb = dchp.tile([128, NDC * S], BF16, tag="dsb", bufs=2)
                for dc in range(NDC):
                    pd = pp.tile([128, S], F32, tag="psum", name=f"pd{nc.next_id()}")
                    for ec in range(n_e):
                        nc.tensor.matmul(
                            pd[:], wts[ec][:, dc * 128:(dc + 1) * 128],
                            act_sb[:, ec * S:(ec + 1) * S],
                            start=(ec == 0), stop=(ec == n_e - 1))
                    nc.scalar.activation(
                        delta_sb[:, dc * S:(dc + 1) * S], pd[:], AF.Copy)
                return delta_sb

            def load_layer_caches(l):
                st = {}
                for b in range(B):
                    kt = cachep.tile([128, KA], BF16, tag=f"kT{b}", bufs=1)
                    nc.sync.dma_start(out=kt[:], in_=kT[l, b])
                    st[("kT", b)] = kt
                    vt = cachep.tile([128, nkc * 128], BF16, tag=f"vTb{b}", bufs=1)
                    nc.sync.dma_start(
                        out=vt[:].rearrange("p (c f) -> p c f", f=128),
                        in_=vT[l, b].rearrange("(c p) f -> p c f", p=128))
                    st[("vTb", b)] = vt
                    for h in range(CH):
                        ck = cachep.tile([128, KT], BF16, tag=f"ckT{b}_{h}", bufs=1)
                        nc.sync.dma_start(out=ck[:], in_=ckT[l, b, h])
                        st[("ckT", b, h)] = ck
                        cv = cachep.tile([128, ntck * 128], BF16, tag=f"cvTb{b}_{h}", bufs=1)
                        nc.sync.dma_start(
                            out=cv[:].rearrange("p (c f) -> p c f", f=128),
                            in_=cvT[l, b, h].rearrange("(c p) f -> p c f", p=128))
                        st[("cvTb", b, h)] = cv
                return st

            def softmax_rows(ps_list, widths, p, mask_sb, mask_off):
                """exp+normalize a row-softmax split over groups.
                ps_list[g]: psum [S, widths[g]]; p: sbuf [S, sum(widths)] out."""
                dparts = []
                col = 0
                for g, ps in enumerate(ps_list):
                    wdt = widths[g]
                    nc.vector.tensor_add(
                        ps[:], ps[:],
                        mask_sb[:, mask_off + col:mask_off + col + wdt])
                    dp_ = smallp.tile([S, 1], F32, tag="denom", bufs=8)
                    nc.scalar.activation(p[:, col:col + wdt], ps[:], AF.Exp,
                                         accum_out=dp_[:])
                    dparts.append(dp_)
                    col += wdt
                denom = dparts[0]
                for dp_ in dparts[1:]:
                    dnew = smallp.tile([S, 1], F32, tag="denom", bufs=8)
                    nc.vector.tensor_add(dnew[:], denom[:], dp_[:])
                    denom = dnew
                invd = smallp.tile([S, 1], F32, tag="invd", bufs=4)
                nc.vector.reciprocal(invd[:], denom[:])
                nc.vector.tensor_scalar_mul(p[:], p[:], invd[:])

            def sa_block(l, b, st):
                invb = stats_invb(b)
                if b == 0:
                    qw_sb = wbig.tile([128, NDC * QH * HD], BF16, tag="wq", bufs=2)
                    nc.scalar.dma_start(
                        out=qw_sb[:].rearrange("p (c e) -> p c e", e=QH * HD),
                        in_=qw[l].rearrange("(c p) e -> p c e", p=128))
                    kv_sb = wbig.tile([128, NDC * 2 * HD], BF16, tag="wkv", bufs=1)
                    nc.scalar.dma_start(
                        out=kv_sb[:].rearrange("p (c e) -> p c e", e=2 * HD),
                        in_=kvw[l].rearrange("(c p) e -> p c e", p=128))
                    st["qw"], st["kv"] = qw_sb, kv_sb
                qw_sb, kv_sb = st["qw"], st["kv"]

                pq0 = pp.tile([128, S], F32, tag="psum", name=f"pq0{nc.next_id()}")
                pq1 = pp.tile([128, S], F32, tag="psum", name=f"pq1{nc.next_id()}")
                pk = pp.tile([128, S], F32, tag="psum", name=f"pk{nc.next_id()}")
                pv = pp.tile([128, S], F32, tag="psum", name=f"pv{nc.next_id()}")
                for i in range(NDC):
                    stt, spp = (i == 0), (i == NDC - 1)
                    qo, ko = i * QH * HD, i * 2 * HD
                    hsl = XB[b][:, i * S:(i + 1) * S]
                    nc.tensor.matmul(pq0[:], qw_sb[:, qo:qo + 128], hsl, start=stt, stop=spp)
                    nc.tensor.matmul(pq1[:], qw_sb[:, qo + 128:qo + 256], hsl, start=stt, stop=spp)
                    nc.tensor.matmul(pk[:], kv_sb[:, ko:ko + 128], hsl, start=stt, stop=spp)
                    nc.tensor.matmul(pv[:], kv_sb[:, ko + 128:ko + 256], hsl, start=stt, stop=spp)

                sn, cs = scaled_sincos(b, invb)
                q_roped = probsp.tile([128, QH * S], BF16, tag="qrope", bufs=2)
                rope_into(q_roped, 0, pq0[:], sn, cs)
                rope_into(q_roped, S, pq1[:], sn, cs)
                rope_into(st[("kT", b)], w, pk[:], sn, cs)

                vsb = probsp.tile([128, S], BF16, tag="vsb", bufs=2)
                nc.vector.tensor_mul(vsb[:], pv[:], invb[:])
                pvt = pp.tile([128, 128], BF16, tag="psum", name=f"pvt{nc.next_id()}")
                nc.tensor.transpose(pvt[:], vsb[:], ident[:])
                r, c0 = w % 128, w // 128
                vtb = st[("vTb", b)]
                if r == 0:
                    nc.vector.tensor_copy(vtb[:, c0 * 128:(c0 + 1) * 128], pvt[:])
                else:
                    nc.vector.tensor_copy(vtb[r:128, c0 * 128:(c0 + 1) * 128],
                                          pvt[0:128 - r, :])
                    nc.vector.tensor_copy(vtb[0:r, (c0 + 1) * 128:(c0 + 2) * 128],
                                          pvt[128 - r:128, :])

                p_tiles = {}
                for h in range(QH):
                    p = probsp.tile([S, KA], BF16, tag="p", bufs=2)
                    ps_list, widths = [], []
                    for g0 in range(0, nkc, 4):
                        wdt = min(4, nkc - g0) * 128
                        ps = pp.tile([S, wdt], F32, tag="psum", name=f"ps{nc.next_id()}")
                        nc.tensor.matmul(
                            ps[:], q_roped[:, h * S:(h + 1) * S],
                            st[("kT", b)][:, g0 * 128:g0 * 128 + wdt],
                            start=True, stop=True)
                        ps_list.append(ps)
                        widths.append(wdt)
                    softmax_rows(ps_list, widths, p, smask_sb, b * KA)
                    p_tiles[h] = p

                pattn = pp.tile([128, QH * S], F32, tag="psum", name=f"pat{nc.next_id()}")
                for j in range(nkc):
                    pT = ptp.tile([128, QH * S], BF16, tag="pT", bufs=2)
                    for h in range(QH):
                        ptps = pp.tile([S, 128], BF16, tag="psum", name=f"ptp{nc.next_id()}")
                        nc.tensor.transpose(
                            ptps[:], p_tiles[h][:, j * 128:(j + 1) * 128], ident[:])
                        nc.vector.tensor_copy(pT[:, h * S:(h + 1) * S], ptps[:])
                    nc.tensor.matmul(
                        pattn[:], st[("vTb", b)][:, j * 128:(j + 1) * 128], pT[:],
                        start=(j == 0), stop=(j == nkc - 1))
                attn_sb = attnp.tile([128, QH * S], BF16, tag="attn")
                nc.scalar.activation(attn_sb[:], pattn[:], AF.Copy)

                if b == 0:
                    wts = []
                    for ec in range(QH):
                        wt = wrow.tile([128, D], BF16, tag="wrow", bufs=2)
                        nc.scalar.dma_start(out=wt[:], in_=ow[l, ec * 128:(ec + 1) * 128, :])
                        wts.append(wt)
                    st["ow"] = wts
                return second_proj(st["ow"], attn_sb, QH)

            def ca_block(l, b, st):
                invb = stats_invb(b)
                if b == 0:
                    cq_sb = wbig.tile([128, NDC * CH * HDC], BF16, tag="wq", bufs=2)
                    nc.scalar.dma_start(
                        out=cq_sb[:].rearrange("p (c e) -> p c e", e=CH * HDC),
                        in_=cqw[l].rearrange("(c p) e -> p c e", p=128))
                    st["cq"] = cq_sb
                cq_sb = st["cq"]
                pcq0 = pp.tile([128, S], F32, tag="psum", name=f"pcq0{nc.next_id()}")
                pcq1 = pp.tile([128, S], F32, tag="psum", name=f"pcq1{nc.next_id()}")
                for i in range(NDC):
                    stt, spp = (i == 0), (i == NDC - 1)
                    qo = i * CH * HDC
                    hsl = XB[b][:, i * S:(i + 1) * S]
                    nc.tensor.matmul(pcq0[:], cq_sb[:, qo:qo + 128], hsl, start=stt, stop=spp)
                    nc.tensor.matmul(pcq1[:], cq_sb[:, qo + 128:qo + 256], hsl, start=stt, stop=spp)
                sn, cs = scaled_sincos(b, invb)
                cq_roped = probsp.tile([128, CH * S], BF16, tag="qrope", bufs=2)
                rope_into(cq_roped, 0, pcq0[:], sn, cs)
                rope_into(cq_roped, S, pcq1[:], sn, cs)

                cattn_sb = attnp.tile([128, CH * S], BF16, tag="attn")
                for h in range(CH):
                    ps = pp.tile([S, KT], F32, tag="psum", name=f"cps{nc.next_id()}")
                    nc.tensor.matmul(
                        ps[:], cq_roped[:, h * S:(h + 1) * S],
                        st[("ckT", b, h)][:], start=True, stop=True)
                    p = probsp.tile([S, KT], BF16, tag="cp", bufs=2)
                    softmax_rows([ps], [KT], p, cmask_sb, b * KT)
                    pcat = pp.tile([128, S], F32, tag="psum", name=f"pca{nc.next_id()}")
                    for j in range(ntck):
                        pT = ptp.tile([S, 128], BF16, tag="cpT", bufs=2)
                        ptps = pp.tile([S, 128], BF16, tag="psum", name=f"ptc{nc.next_id()}")
                        nc.tensor.transpose(
                            ptps[:], p[:, j * 128:(j + 1) * 128], ident[:])
                        nc.vector.tensor_copy(pT[:], ptps[:])
                        nc.tensor.matmul(
                            pcat[:], st[("cvTb", b, h)][:, j * 128:(j + 1) * 128], pT[:],
                            start=(j == 0), stop=(j == ntck - 1))
                    nc.scalar.activation(cattn_sb[:, h * S:(h + 1) * S],
                                         pcat[:], AF.Copy)

                if b == 0:
                    wts = []
                    for ec in range(CH):
                        wt = wrow.tile([128, D], BF16, tag="wrow", bufs=2)
                        nc.scalar.dma_start(out=wt[:], in_=cow[l, ec * 128:(ec + 1) * 128, :])
                        wts.append(wt)
                    st["cow"] = wts
                return second_proj(st["cow"], cattn_sb, CH)

            def mlp_block_b(l, b, st):
                """MLP for one batch; weights loaded at b==0 stay resident."""
                invb = stats_invb(b)
                WW = 2
                if b == 0:
                    gts, uts = [], []
                    for wv in range(NFC // WW):
                        gt = wmlp.tile([128, NDC * WW * 128], BF16,
                                       tag=f"wg{wv}", bufs=1)
                        nc.scalar.dma_start(
                            out=gt[:].rearrange("p (c f) -> p c f", f=WW * 128),
                            in_=wgw[l, :, wv * WW * 128:(wv + 1) * WW * 128].rearrange(
                                "(c p) f -> p c f", p=128))
                        ut = wmlp.tile([128, NDC * WW * 128], BF16,
                                       tag=f"wu{wv}", bufs=1)
                        nc.scalar.dma_start(
                            out=ut[:].rearrange("p (c f) -> p c f", f=WW * 128),
                            in_=wuw[l, :, wv * WW * 128:(wv + 1) * WW * 128].rearrange(
                                "(c p) f -> p c f", p=128))
                        gts.append(gt)
                        uts.append(ut)
                    st["wg"], st["wu"] = gts, uts
                    wd_sb = []
                    for dhalf in range(2):
                        for fg in range(2):
                            t = wdp.tile([128, 4 * (D // 2)], BF16,
                                         tag=f"wd{dhalf}_{fg}", bufs=1)
                            nc.scalar.dma_start(
                                out=t[:].rearrange("p (c f) -> p c f", f=D // 2),
                                in_=wdw[l, fg * 512:(fg + 1) * 512,
                                        dhalf * (D // 2):(dhalf + 1) * (D // 2)].rearrange(
                                    "(c p) f -> p c f", p=128))
                            wd_sb.append(t)
                    st["wd"] = wd_sb
                gts, uts, wd_sb = st["wg"], st["wu"], st["wd"]

                m_tiles = []
                for wv in range(NFC // WW):
                    gt, ut = gts[wv], uts[wv]
                    for k in range(WW):
                        pg = pp.tile([128, S], F32, tag="psum", name=f"pg{nc.next_id()}")
                        pu = pp.tile([128, S], F32, tag="psum", name=f"pu{nc.next_id()}")
                        for i in range(NDC):
                            stt, spp = (i == 0), (i == NDC - 1)
                            co = i * WW * 128 + k * 128
                            hsl = XB[b][:, i * S:(i + 1) * S]
                            nc.tensor.matmul(pg[:], gt[:, co:co + 128],
                                             hsl, start=stt, stop=spp)
                            nc.tensor.matmul(pu[:], ut[:, co:co + 128],
                                             hsl, start=stt, stop=spp)
                        sgs = mtilep.tile([128, S], BF16, tag="sgs", bufs=2)
                        nc.vector.tensor_mul(sgs[:], pg[:], invb[:])
                        sg = mtilep.tile([128, S], BF16, tag="sg")
                        nc.scalar.activation(sg[:], sgs[:], AF.Silu)
                        ub = mtilep.tile([128, S], BF16, tag="ub")
                        nc.vector.tensor_mul(ub[:], pu[:], invb[:])
                        m = mtilep.tile([128, S], BF16, tag="m", bufs=12)
                        nc.vector.tensor_mul(m[:], sg[:], ub[:])
                        m_tiles.append(m)

                delta = dchp.tile([128, NDC * S], BF16, tag="dsb", bufs=2,
                                  name=f"dmlp{l}_{b}")
                for dhalf in range(2):
                    for dc8 in range(NDC // 2):
                        dc = dhalf * (NDC // 2) + dc8
                        pd = pp.tile([128, S], F32, tag="psum", name=f"pdm{nc.next_id()}")
                        for fc in range(NFC):
                            t = wd_sb[dhalf * 2 + fc // 4]
                            co = (fc % 4) * (D // 2) + dc8 * 128
                            nc.tensor.matmul(
                                pd[:], t[:, co:co + 128],
                                m_tiles[fc][:], start=(fc == 0), stop=(fc == NFC - 1))
                        nc.scalar.activation(
                            delta[:, dc * S:(dc + 1) * S], pd[:], AF.Copy)
                return delta

            # ================= layers (batch-staggered pipeline) =========
            for l in range(L):
                st = load_layer_caches(l)
                # SA
                for b in range(B):
                    if l > 0:
                        residual(3 * (l - 1) + 2, b)
                    d = sa_block(l, b, st)
                    ar_issue(3 * l + 0, b, d)
                # CA
                for b in range(B):
                    residual(3 * l + 0, b)
                    d = ca_block(l, b, st)
                    ar_issue(3 * l + 1, b, d)
                # MLP (batch-sequential: AR of b0 hides under b1 compute)
                for b in range(B):
                    residual(3 * l + 1, b)
                    d = mlp_block_b(l, b, st)
                    ar_issue(3 * l + 2, b, d)

            # ================ final norm + output ================
            nc.vector.tensor_scalar_mul(warm_sb[:], warm_sb[:], 0.0)
            for b in range(B):
                residual(3 * (L - 1) + 2, b, last=True)
                nw = pp.tile([1, 4 * S], F32, tag="psum", name=f"nsf{b}")
                for g in range(4):
                    sq = sqp.tile([128, 4 * S], BF16, tag="sq", bufs=1)
                    nc.scalar.activation(sq[:], X[b][:, g * 4 * S:(g + 1) * 4 * S],
                                         AF.Square)
                    nc.tensor.matmul(nw[:], ones_col[:], sq[:],
                                     start=(g == 0), stop=(g == 3))
                nwsb = smallp.tile([1, 4 * S], F32, tag="nwsb", bufs=1)
                nc.vector.tensor_copy(nwsb[:], nw[:])
                t0 = smallp.tile([1, S], F32, tag="nfold", bufs=3)
                t1 = smallp.tile([1, S], F32, tag="nfold", bufs=3)
                nc.vector.tensor_add(t0[:], nwsb[:, 0:S], nwsb[:, S:2 * S])
                nc.vector.tensor_add(t1[:], nwsb[:, 2 * S:3 * S], nwsb[:, 3 * S:4 * S])
                nstat = smallp.tile([1, S], F32, tag="nfold", bufs=3)
                nc.vector.tensor_add(nstat[:], t0[:], t1[:])
                sd = smallp.tile([1, S], F32, tag="sd")
                nc.scalar.activation(sd[:], nstat[:], AF.Sqrt,
                                     bias=eps_t[:], scale=1.0 / D)
                inv = smallp.tile([1, S], F32, tag="inv")
                nc.vector.reciprocal(inv[:], sd[:])
                invps = pp.tile([128, S], F32, tag="psum", name=f"inf{b}")
                nc.tensor.matmul(invps[:], ones_row[:], inv[:], start=True, stop=True)
                invb = invp.tile([128, S], F32, tag="invb", bufs=2)
                nc.vector.tensor_copy(invb[:], invps[:])
                for g in range(4):
                    sl = slice(g * 4 * S, (g + 1) * 4 * S)
                    nc.vector.tensor_mul(X[b][:, sl], X[b][:, sl],
                                         invb[:].rearrange("p (o s) -> p o s", o=1)
                                         .to_broadcast((128, 4, S)))
                    nc.vector.tensor_mul(
                        X[b][:, sl].rearrange("p (c s) -> p c s", s=S),
                        X[b][:, sl].rearrange("p (c s) -> p c s", s=S),
                        fnw_sb[:, g * 4:(g + 1) * 4].rearrange("p (c o) -> p c o", o=1)
                        .to_broadcast((128, 4, S)))
                if b == 0:
                    nc.vector.tensor_add(X[b][0:1, 0:16], X[b][0:1, 0:16],
                                         warm_sb[:])
                nc.sync.dma_start(
                    out=out_ext[:, b * S:(b + 1) * S].rearrange(
                        "(c p) s -> p c s", p=128),
                    in_=X[b][:].rearrange("p (c s) -> p c s", s=S))

    nc.finalize()
    return nc


# ---------------------------------------------------------------- host prep
def _prep_in_maps(inputs):
    f32 = np.float32
    x = inputs["x"].astype(f32)                      # (B, D, 1, S)
    positions = inputs["positions"]
    w = int(np.asarray(inputs["kv_write_index"]).reshape(-1)[0])
    self_attn_mask = inputs["self_attn_mask"].astype(f32)  # (B,1,S,A)
    enc_len = np.asarray(inputs["encoder_lengths"]).reshape(B)

    sa_n = inputs["sa_norm_w"].astype(f32)[:, :, None]     # (L, D, 1)
    ca_n = inputs["ca_norm_w"].astype(f32)[:, :, None]
    mlp_n = inputs["mlp_norm_w"].astype(f32)[:, :, None]
    scale = 1.0 / np.sqrt(HD).astype(f32)
    cscale = 1.0 / np.sqrt(HDC).astype(f32)

    qw = (inputs["q_w"] * sa_n * scale).astype(BF)         # (L, D, HQ*HD)
    kw = (inputs["k_w"] * sa_n).astype(BF)
    vw = (inputs["v_w"] * sa_n).astype(BF)
    ow = inputs["o_w"].astype(BF)                          # (L, HQ*HD, D)
    cqw = (inputs["cq_w"] * ca_n * cscale).astype(BF)
    cow = inputs["co_w"].astype(BF)
    wgw = (inputs["wg_w"] * mlp_n).astype(BF)
    wuw = (inputs["wu_w"] * mlp_n).astype(BF)
    wdw = inputs["wd_w"].astype(BF)

    # --- masked-chunk classification -------------------------------------
    # self-attn: chunk c of the A axis is skippable if fully masked for all
    # queries in both batches; visible chunks form a prefix for causal masks.
    m = self_attn_mask[:, 0]                               # (B, S, A)
    ch = m.reshape(B, S, A // 128, 128)
    full_masked = (ch < -1e3).all(axis=(0, 1, 3))          # (A//128,)
    keep = np.nonzero(~full_masked)[0]
    if len(keep) == 0:
        nkc = A // 128
        m = m + 3e4
    else:
        nkc = int(keep[-1]) + 1
    # ensure the KV write region is covered
    nkc = max(nkc, (w + S + 127) // 128)
    nkc = min(nkc, A // 128)
    KA = nkc * 128

    # cross-attn: valid keys are t < enc_len (prefix); fully-masked batch
    # falls back to all chunks with a shifted (all-zero) mask == uniform.
    if (enc_len <= 0).any():
        ntck = T // 128
    else:
        ntck = int((enc_len.max() + 127) // 128)
    KT = ntck * 128

    k_cache = inputs["k_cache"].reshape(L, B, HKV, A, HD)
    v_cache = inputs["v_cache"].reshape(L, B, HKV, HD, A)
    ck = inputs["ck_cache"].reshape(L, B, HC, T, HDC)
    cv = inputs["cv_cache"].reshape(L, B, HC, HDC, T)
    kTf = np.ascontiguousarray(
        k_cache.transpose(0, 1, 2, 4, 3)[:, :, :, :, :KA]).astype(BF)
    vTf = np.ascontiguousarray(
        v_cache.transpose(0, 1, 2, 4, 3)[:, :, :, :KA, :]).astype(BF)
    ckTf = np.ascontiguousarray(
        ck.transpose(0, 1, 2, 4, 3)[:, :, :, :, :KT]).astype(BF)
    cvTf = np.ascontiguousarray(
        cv.transpose(0, 1, 2, 4, 3)[:, :, :KT, :]).astype(BF)

    inv_freq = 1.0 / (10000.0 ** (np.arange(0, HD, 2, dtype=f32) / HD))
    ang = positions.astype(f32)[:, None, :] * inv_freq[None, :, None]   # (B,64,S)
    sinT = np.ascontiguousarray(np.sin(ang).transpose(1, 0, 2).reshape(64, B * S)).astype(f32)
    cosT = np.ascontiguousarray(np.cos(ang).transpose(1, 0, 2).reshape(64, B * S)).astype(f32)

    smask = np.ascontiguousarray(
        m[:, :, :KA].transpose(1, 0, 2).reshape(S, B * KA)).astype(BF)
    t_idx = np.arange(KT)
    cm = np.where(t_idx[None, :] < enc_len[:, None], 0.0, NEG).astype(f32)
    for b in range(B):
        if enc_len[b] <= 0:
            cm[b] += 3e4
    cmask = np.ascontiguousarray(
        np.broadcast_to(cm.reshape(1, B * KT), (S, B * KT))).astype(BF)

    x_in = np.ascontiguousarray(
        x[:, :, 0, :].transpose(1, 0, 2).reshape(D, B * S)).astype(f32)
    fnw = np.ascontiguousarray(
        inputs["final_norm_w"].astype(f32).reshape(NDC, 128).T)

    in_maps = []
    for c in range(NCORES):
        qh = slice(2 * c * HD, (2 * c + 2) * HD)
        kvh = c // 2
        ffs = slice(c * FFS, (c + 1) * FFS)
        in_maps.append({
            "x_in": x_in,
            "qw": np.ascontiguousarray(qw[:, :, qh]),
            "kvw": np.ascontiguousarray(np.concatenate(
                [kw[:, :, kvh * HD:(kvh + 1) * HD],
                 vw[:, :, kvh * HD:(kvh + 1) * HD]], axis=2)),
            "ow": np.ascontiguousarray(ow[:, qh, :]),
            "cqw": np.ascontiguousarray(cqw[:, :, qh]),
            "cow": np.ascontiguousarray(cow[:, qh, :]),
            "wgw": np.ascontiguousarray(wgw[:, :, ffs]),
            "wuw": np.ascontiguousarray(wuw[:, :, ffs]),
            "wdw": np.ascontiguousarray(wdw[:, ffs, :]),
            "kT": np.ascontiguousarray(kTf[:, :, kvh]),
            "vT": np.ascontiguousarray(vTf[:, :, kvh]),
            "ckT": np.ascontiguousarray(ckTf[:, :, 2 * c:2 * c + 2]),
            "cvT": np.ascontiguousarray(cvTf[:, :, 2 * c:2 * c + 2]),
            "sinT": sinT, "cosT": cosT,
            "smask": smask, "cmask": cmask,
            "fnw": fnw,
        })
    return in_maps, w, nkc, ntck


def kernel(**inputs):
    global _exec_time_ns
    in_maps, w, nkc, ntck = _prep_in_maps(inputs)
    nc = build_graph(w, nkc, ntck)
    trace = bool(int(os.environ.get("BASS_KERNEL_TRACE", "0")))
    res = run_bass_kernel_spmd(nc, in_maps, list(range(NCORES)), trace=trace)
    _exec_time_ns = res.exec_time_ns
    out = np.asarray(res.results[0]["out"])          # [D, B*S] f32
    out = out.reshape(D, B, S).transpose(1, 0, 2)[:, :, None, :]
    return np.ascontiguousarray(out.astype(np.float32))
